# revision 47
# baseline (speedup 1.0000x reference)
"""Trainium2 Bass kernel for nn_BoundaryModule_38422777430159.

Reference computation (B=4, C=256, T=256, N=10, D=40, DIM0=512, DIM1=128):
  x1 = sample(feature)            # (B,C,N,D,T) via (T, N*D*T) smp matmul
  x2 = leaky(einsum('bcndt,ocn->bodt', x1, w0) + b0)
  x3 = leaky(w1 @ x2 + b1)        # 1x1 conv
  x4 = leaky(conv3x3(x3, w2) + b2)
  out = sigmoid(w3 @ x4 + b3)     # (B, D, T)

Device strategy (8 cores SPMD; core = (b = i//2, t-half th = i%2), 1-col halo):

The smp matrix is linear interpolation: each output column (n,d,t) touches
<=2 adjacent tau rows of G_n = w0_n.T @ feature.  The dense contraction
over (n,tau)=2560 rows is therefore ~99% zeros.  This kernel packs, for
each (13 t x 20 d) output tile, exactly the tau-bands it needs into a few
dense 128-row chunks:

  A:    A[n,win] = feat_win.T @ w0_n  (bf16, 3 tau-windows of 128/n)
        -> DMA to a DRAM scratch (3840 rows of [512], +1 row = b0)
  pack: per chunk, one gpsimd indirect-DMA gather (idx per partition,
        host-computed from the smp nonzero pattern); row 0 = b0 row so the
        block0 bias rides the matmul
  B:    per tile: ~2.5 chunks x 4 o-blocks of [128,260] bf16 matmuls
        against host-packed W' slices; Act-engine Prelu -> x2 (bf16)
  C:    x3 = w1 @ x2 (+ b1 x colmask via a rank-1 matmul so halo columns
        stay exactly 0), Prelu written strided into the conv pad buffer
  D/E:  3x3 conv (bf16) + sigmoid(w3 @ x4 + b3) as before.

All matmuls bf16 (1 cyc/row at any free size); PSUM accumulates fp32.
"""
import os
import sys

for _p in ("/opt/trn_rl_repo", "/root/.axon_site/_ro/trn_rl_repo"):
    if os.path.isdir(_p) and _p not in sys.path:
        sys.path.append(_p)

import numpy as np
import ml_dtypes

import concourse.bass as bass
import concourse.tile as tile
from concourse import mybir
from concourse.bass_utils import run_bass_kernel_spmd

T = 256
N = 10
D = 40
B = 4
C_IN = 256
DIM0 = 512
DIM1 = 128

TW = 130            # t-window incl 1-col halo each side
DI, DD = 13, 20     # stage-B tile: 13 t-cols x 20 d-rows
FW = DI * DD        # 260 matmul columns per tile
NIT, NDT = TW // DI, D // DD   # 10 x 2 tiles
KG = 3              # i-tiles grouped per chunk-set
WIN = 3             # tau windows per n in stage A
TAU0 = -64          # window 0 starts at absolute tau = -64
AW = WIN * 128      # 384 feat' columns
NAROW = N * AW      # 3840 real A rows; row 3840 = b0
CAP = 127           # chunk rows 1..127 carry data; row 0 = b0
DCH = 3             # conv d-rows per psum group
NDCH = (D + DCH - 1) // DCH  # 14

F32 = mybir.dt.float32
BF16 = mybir.dt.bfloat16
I32 = mybir.dt.int32
BF = ml_dtypes.bfloat16


def _legalize_waits(nc, limit=1):
    """This walrus build allows a single embedded sync wait per real
    instruction; move the excess onto standalone NoOp wait-carriers."""
    moved = 0
    for f in nc.m.functions:
        for bb in f.blocks:
            il = bb.instructions
            out = []
            changed = False
            for inst in il:
                si = inst.sync_info
                ty = type(inst).__name__
                if (si and si.on_wait and len(si.on_wait) > limit
                        and ty not in ("InstEventSemaphore", "InstNoOp")):
                    keep = si.on_wait[-limit:]
                    for w in si.on_wait[:-limit]:
                        out.append(mybir.InstNoOp(
                            name=f"waitnop-{nc.next_id()}",
                            sync_info=mybir.SyncInfo(on_wait=[w], on_update=[]),
                            bass_nofuse=True,
                            engine=inst.engine,
                        ))
                        moved += 1
                    inst.sync_info = mybir.SyncInfo(
                        on_wait=keep, on_update=si.on_update)
                    changed = True
                out.append(inst)
            if changed:
                bb.instructions = out
    return moved


# ---------------------------------------------------------------------------
# host-side geometry: which (n, tau) rows each stage-B tile needs, grouped
# into shared chunk-sets; identical program structure for every core.
# ---------------------------------------------------------------------------

def _tile_cols(it, th):
    """absolute i for the 13 columns of i-tile `it` on half `th` (may be
    outside [0, T): those columns are pad)."""
    return [it * DI + il - 1 + 128 * th for il in range(DI)]


def _geometry(nzmask):
    """nzmask: [T(tau), N, D, T(i)] bool of the smp matrix.
    Returns the program structure + per-th packing:
      groups: list of dicts with tiles, nch, and per-th
              (chunk_rows[th][c] = list of (n, tau) or None) ...
      tiles:  dict (it,dt) -> (group_idx, s0, s1, b0_slot_local)
    """
    # per (th, it): valid i list and column index
    tile_rows = {}
    for th in range(2):
        for it in range(NIT):
            cols = _tile_cols(it, th)
            valid = [i for i in cols if 0 <= i < T]
            for dt in range(NDT):
                if valid:
                    sub = nzmask[:, :, dt * DD:(dt + 1) * DD, :][:, :, :, valid]
                    tn = np.argwhere(sub.any(axis=(2, 3)))  # (tau, n)
                    rows = set((int(n_), int(t_)) for t_, n_ in tn)
                else:
                    rows = set()
                tile_rows[(th, it, dt)] = rows

    groups = []
    tiles = {}
    chunk_base = 0
    for dt in range(NDT):
        for g0 in range(0, NIT, KG):
            its = list(range(g0, min(g0 + KG, NIT)))
            per_th = []
            spans = [{}, {}]
            for th in range(2):
                sets = [tile_rows[(th, it, dt)] for it in its]
                allrows = sorted(set().union(*sets))
                if allrows:
                    use = {r: [k for k, s in enumerate(sets) if r in s]
                           for r in allrows}
                    allrows.sort(key=lambda r: (float(np.mean(use[r])), r))
                pos = {r: j for j, r in enumerate(allrows)}
                nch = max(1, -(-len(allrows) // CAP))
                chunk_rows = []
                for c in range(nch):
                    chunk_rows.append(allrows[c * CAP:(c + 1) * CAP])
                per_th.append(chunk_rows)
                for it, s in zip(its, sets):
                    if s:
                        ps = [pos[r] for r in s]
                        spans[th][it] = (min(ps) // CAP, max(ps) // CAP)
            nch = max(len(per_th[0]), len(per_th[1]))
            for it in its:
                sp = [spans[th][it] for th in range(2) if it in spans[th]]
                if sp:
                    s0 = min(a for a, b in sp)
                    s1 = max(b for a, b in sp)
                else:
                    s0 = s1 = 0
                tiles[(it, dt)] = (len(groups), s0, s1)
            groups.append(dict(dt=dt, its=its, nch=nch, per_th=per_th,
                               chunk_base=chunk_base))
            chunk_base += nch
    return groups, tiles, chunk_base


def _build_core_data(smp, geom, th):
    """Per-t-half gather indices and packed W' slices (shared across b)."""
    groups, tiles, nch_total = geom
    # smp padded in i: index ip = i+1 in [0, 258)
    smp_pad = np.zeros((T, N, D, T + 2), dtype=np.float32)
    smp_pad[:, :, :, 1:T + 1] = smp

    idx = np.full((128, nch_total), NAROW, dtype=np.int32)  # default: b0 row
    for g in groups:
        rows_c = g["per_th"][th] if th < len(g["per_th"]) else []
        for c in range(g["nch"]):
            rows = rows_c[c] if c < len(rows_c) else []
            for j, (n_, tau) in enumerate(rows):
                idx[1 + j, g["chunk_base"] + c] = n_ * AW + (tau - TAU0)

    # slot list in program emission order: for each tile (dt-major), its
    # slots s0..s1
    slots = []
    for dt in range(NDT):
        for it in range(NIT):
            gi, s0, s1 = tiles[(it, dt)]
            for s in range(s0, s1 + 1):
                slots.append((it, dt, gi, s, s == s0))
    nslot = len(slots)

    wp = np.zeros((nslot, 128, FW), dtype=BF)
    ip_cols = {}
    for it in range(NIT):
        ip_cols[it] = np.array([min(max(i + 1, 0), T + 1)
                                for i in _tile_cols(it, th)])
    for si, (it, dt, gi, s, isfirst) in enumerate(slots):
        g = groups[gi]
        rows_c = g["per_th"][th]
        rows = rows_c[s] if s < len(rows_c) else []
        dsl = slice(dt * DD, (dt + 1) * DD)
        if rows:
            ns = np.array([r[0] for r in rows])
            taus = np.array([r[1] for r in rows])
            vals = smp_pad[taus, ns][:, dsl, :][:, :, ip_cols[it]]
            wp[si, 1:1 + len(rows)] = vals.reshape(len(rows), FW).astype(BF)
        if isfirst:
            mask = np.array([1.0 if 0 <= i < T else 0.0
                             for i in _tile_cols(it, th)], dtype=np.float32)
            wp[si, 0] = np.tile(mask, DD).astype(BF)

    # colmask for the b1 rank-1 matmul, tiled per i-tile: [1, NIT*FW]
    cm = np.zeros((1, NIT * FW), dtype=BF)
    for it in range(NIT):
        mask = np.array([1.0 if 0 <= i < T else 0.0
                         for i in _tile_cols(it, th)], dtype=np.float32)
        cm[0, it * FW:(it + 1) * FW] = np.tile(mask, DD).astype(BF)
    return idx, wp, cm, slots


# ---------------------------------------------------------------------------
# program
# ---------------------------------------------------------------------------

def _build_program(geom, trunc=None):
    # trunc: debug levels 'a' (stage A only), 'pack', 'bc' (no conv)
    groups, tiles, nch_total = geom
    slots = []
    for dt in range(NDT):
        for it in range(NIT):
            gi, s0, s1 = tiles[(it, dt)]
            for s in range(s0, s1 + 1):
                slots.append((it, dt, gi, s))
    nslot = len(slots)
    WMAX = max(s1 - s0 + 1 for _, s0, s1 in tiles.values())

    nc = bass.Bass(trn_type="TRN2", num_swdge_queues=4)
    PRELU = mybir.ActivationFunctionType.Prelu
    SIG = mybir.ActivationFunctionType.Sigmoid

    feat_d = nc.dram_tensor("feat", [128, 2, AW], BF16, kind="ExternalInput")
    w0_d = nc.dram_tensor("w0t", [128, N, 2, DIM0], BF16, kind="ExternalInput")
    wp_d = nc.dram_tensor("wp", [nslot, 128, FW], BF16, kind="ExternalInput")
    idx_d = nc.dram_tensor("gidx", [128, nch_total], I32, kind="ExternalInput")
    # w1 (4x128) | w2 (9x128) | w3 (1 col) along the free dim
    wsm_d = nc.dram_tensor("wsm", [128, 14 * DIM1 + 1], BF16,
                           kind="ExternalInput")
    b0r_d = nc.dram_tensor("b0r", [1, DIM0], BF16, kind="ExternalInput")
    # per-partition columns: b1 | b2 | b3(row0) | halo maskL | maskR
    b123_d = nc.dram_tensor("b123", [128, 5], F32, kind="ExternalInput")
    a_d = nc.dram_tensor("adram", [NAROW + 1, DIM0], BF16, kind="Internal")
    out_d = nc.dram_tensor("out", [1, D * TW], F32, kind="ExternalOutput")

    with tile.TileContext(nc) as tc:
        with (
            tc.tile_pool(name="inp", bufs=1) as inp,
            tc.tile_pool(name="asb", bufs=4) as asb,
            tc.tile_pool(name="apk", bufs=1) as apk,
            tc.tile_pool(name="wst", bufs=4) as wst,
            tc.tile_pool(name="x2p", bufs=2) as x2p,
            tc.tile_pool(name="x3p", bufs=1) as x3p,
            tc.tile_pool(name="x4p", bufs=2) as x4p,
            tc.tile_pool(name="outp", bufs=1) as outp,
            tc.tile_pool(name="psb", bufs=1, space="PSUM") as psb,
            tc.tile_pool(name="psg", bufs=2, space="PSUM") as psg,
        ):
            # ---- input DMAs (few, large; feat + first w0 group first) ----
            ft = inp.tile([128, 2, AW], BF16, tag="f", name="feat_sb")
            nc.sync.dma_start(ft[:], feat_d[:])
            feat = [ft[:, c] for c in range(2)]
            w0sb = inp.tile([128, N, 2, DIM0], BF16, tag="w0", name="w0_sb")
            NG = [(0, 3), (3, 6), (6, 9), (9, 10)]
            for gi_, (n0, n1) in enumerate(NG):
                eng = (nc.sync, nc.scalar)[gi_ % 2]
                eng.dma_start(w0sb[:, n0:n1], w0_d[:, n0:n1])
            w0t = [[w0sb[:, n, c] for c in range(2)] for n in range(N)]
            b0r_t = inp.tile([1, DIM0], BF16, tag="b0r", name="b0r_sb")
            nc.scalar.dma_start(b0r_t[:], b0r_d[:])
            # b0 row of the A scratch
            nc.scalar.dma_start(a_d[NAROW:NAROW + 1, :], b0r_t[:])
            idx_t = inp.tile([128, nch_total], I32, tag="idx", name="gidx_sb")
            nc.sync.dma_start(idx_t[:], idx_d[:])
            wsm = inp.tile([128, 14 * DIM1 + 1], BF16, tag="wsm", name="wsm_sb")
            nc.scalar.dma_start(wsm[:], wsm_d[:])
            w1t = [wsm[:, c * DIM1:(c + 1) * DIM1] for c in range(4)]
            w2t = [wsm[:, (4 + j) * DIM1:(5 + j) * DIM1] for j in range(9)]
            w3t = wsm[:, 13 * DIM1:13 * DIM1 + 1]
            b123 = inp.tile([128, 5], F32, tag="b123", name="b123_sb")
            nc.sync.dma_start(b123[:], b123_d[:])
            b1t = b123[:, 0:1]
            b2t = b123[:, 1:2]
            b3t = b123[0:1, 2:3]
            mlr_t = inp.tile([128, 2], BF16, tag="mlr", name="mlr_sb")
            nc.vector.tensor_copy(mlr_t[:], b123[:, 3:5])

            # ---- PE warm-up: keep the PE busy (and its p-state ramped)
            # while the feat/w0 DMAs land and later while the gathers run.
            warm = inp.tile([128, DIM0], BF16, tag="wm", name="warm_sb")
            nc.vector.memset(warm[:].bitcast(mybir.dt.uint16), 0)
            wps = psb.tile([1, DIM0], F32, tag="b3", name="warm_ps")

            def warm_mm(k, free=64):
                nc.tensor.matmul(wps[:, 0:free], warm[:, 0:1], warm[:, 0:free],
                                 start=True, stop=True)

            for k in range(10):
                warm_mm(k)

            # ---- stage A: A[n,win] = feat_win.T @ w0_n -> adram ----
            for n in range(N):
                a_n = asb.tile([128, WIN, DIM0], BF16, tag=f"a{n % 4}",
                               name=f"a{n}")
                for win in range(WIN):
                    k = n * WIN + win
                    ps = psb.tile([128, DIM0], F32, tag=f"b{k % 4}",
                                  name=f"psa{n}_{win}")
                    for c in range(2):
                        nc.tensor.matmul(
                            ps[:],
                            feat[c][:, win * 128:(win + 1) * 128],
                            w0t[n][c][:],
                            start=(c == 0), stop=(c == 1),
                        )
                    if (n * WIN + win) % 2:
                        nc.vector.tensor_copy(a_n[:, win], ps[:])
                    else:
                        nc.scalar.activation(
                            a_n[:, win], ps[:],
                            mybir.ActivationFunctionType.Copy,
                            bias=0.0, scale=1.0)
                nc.sync.dma_start(
                    a_d[n * AW:(n + 1) * AW, :].rearrange(
                        "(w p) e -> p w e", p=128),
                    a_n[:])

            # ---- pack: one indirect gather per chunk ----
            apack = []
            if trunc != 'a':
                for ch in range(nch_total):
                    g = apk.tile([128, DIM0], BF16, tag=f"ap{ch}", name=f"ap{ch}")
                    h = nc.gpsimd.indirect_dma_start(
                        out=g[:], out_offset=None, in_=a_d[:],
                        in_offset=bass.IndirectOffsetOnAxis(
                            ap=idx_t[:, ch:ch + 1], axis=0))
                    q = ch % 4
                    if q:
                        h.ins.queue = f"qPoolDynamic{q}"
                    apack.append(g)

            if trunc in ('a', 'pack'):
                out_sb = outp.tile([1, D * TW], F32, tag="os", name="out_sb")
                nc.vector.memset(out_sb[:], 0.0)
                for ch in range(len(apack)):
                    nc.vector.tensor_copy(out_sb[:, ch:ch + 1],
                                          apack[ch][0:1, 0:1].bitcast(BF16)[:, 0:1] if False
                                          else apack[ch][0:1, 0:1])
                nc.scalar.dma_start(out_d[:], out_sb[:])

            # ---- stages B+C per tile; conv chunks interleaved per d-block --
            emit_rest = trunc not in ('a', 'pack')
            pad = x3p.tile([128, D + 2, TW + 2], BF16, tag="pad", name="padbuf")
            if emit_rest:
                nc.vector.memset(pad[:].bitcast(mybir.dt.uint16), 0)
                out_sb = outp.tile([1, D * TW], F32, tag="os", name="out_sb")
            x4cs = [None] * NDCH

            def stage_bc(it, dt):
                gi, s0, s1 = tiles[(it, dt)]
                g = groups[gi]
                ns = s1 - s0 + 1
                si0 = slot_id[(it, dt, s0)]
                wt = wst.tile([128, WMAX, FW], BF16, tag="w",
                              name=f"wp{it}_{dt}")
                eng = (nc.sync, nc.scalar)[(it + dt) % 2]
                eng.dma_start(wt[:, 0:ns],
                              wp_d[si0:si0 + ns].transpose((1, 0, 2)))
                acc = [psb.tile([128, FW], F32, tag=f"b{ob}",
                                name=f"psb{it}_{dt}_{ob}") for ob in range(4)]
                for s in range(s0, s1 + 1):
                    ch = g["chunk_base"] + s
                    for ob in range(4):
                        nc.tensor.matmul(
                            acc[ob][:],
                            apack[ch][:, ob * 128:(ob + 1) * 128],
                            wt[:, s - s0],
                            start=(s == s0), stop=(s == s1),
                        )
                x2c = []
                for ob in range(4):
                    yt = x2p.tile([128, FW], BF16, tag=f"x2_{ob}",
                                  name=f"x2_{it}_{dt}_{ob}")
                    if ob < 2:
                        nc.scalar.activation(yt[:], acc[ob][:], PRELU,
                                             bias=0.0, scale=1.0, alpha=0.01)
                    else:
                        nc.vector.tensor_copy(yt[:], acc[ob][:])
                        nc.vector.scalar_tensor_tensor(
                            yt[:], yt[:], 0.01, yt[:],
                            mybir.AluOpType.mult, mybir.AluOpType.max)
                    x2c.append(yt)
                psc = psg.tile([128, FW], F32, tag="g", name=f"psc{it}_{dt}")
                for ob in range(4):
                    nc.tensor.matmul(psc[:], w1t[ob][:], x2c[ob][:],
                                     start=(ob == 0), stop=(ob == 3))
                nc.scalar.activation(
                    pad[:, 1 + dt * DD:1 + (dt + 1) * DD,
                        1 + it * DI:1 + (it + 1) * DI],
                    psc[:].rearrange("p (d t) -> p d t", d=DD),
                    PRELU, bias=b1t, scale=1.0, alpha=0.01)
                # exact zero-padding: the per-core invalid halo column
                # (t = -1 for th0, t = 256 for th1) is scaled by a 0/1 mask
                if it == 0:
                    nc.vector.tensor_tensor(
                        pad[:, 1 + dt * DD:1 + (dt + 1) * DD, 1],
                        pad[:, 1 + dt * DD:1 + (dt + 1) * DD, 1],
                        mlr_t[:, 0:1].to_broadcast([128, DD]),
                        mybir.AluOpType.mult)
                if it == NIT - 1:
                    nc.vector.tensor_tensor(
                        pad[:, 1 + dt * DD:1 + (dt + 1) * DD, TW],
                        pad[:, 1 + dt * DD:1 + (dt + 1) * DD, TW],
                        mlr_t[:, 1:2].to_broadcast([128, DD]),
                        mybir.AluOpType.mult)

            slot_id = {}
            k = 0
            for dt in range(NDT):
                for it in range(NIT):
                    gi, s0, s1 = tiles[(it, dt)]
                    for s in range(s0, s1 + 1):
                        slot_id[(it, dt, s)] = k
                        k += 1

            def stage_d(dc):
                d0 = dc * DCH
                nd = min(DCH, D - d0)
                fw = nd * TW
                psd = psg.tile([128, DCH * TW], F32, tag="d", name=f"psd{dc}")
                for j in range(9):
                    dy, dx = j // 3, j % 3
                    nc.tensor.matmul(
                        psd[:, 0:fw],
                        w2t[j][:],
                        pad[:, d0 + dy:d0 + dy + nd, dx:dx + TW],
                        start=(j == 0), stop=(j == 8),
                    )
                x4c = x4p.tile([128, DCH * TW], BF16, tag=f"x4_{dc}",
                               name=f"x4_{dc}")
                nc.scalar.activation(x4c[:, 0:fw], psd[:, 0:fw], PRELU,
                                     bias=b2t[:], scale=1.0, alpha=0.01)
                x4cs[dc] = x4c

            def stage_e(dc):
                d0 = dc * DCH
                fw = min(DCH, D - d0) * TW
                pse = psg.tile([1, DCH * TW], F32, tag="g", name=f"pse{dc}")
                nc.tensor.matmul(pse[:, 0:fw], w3t[:], x4cs[dc][:, 0:fw],
                                 start=True, stop=True)
                nc.scalar.activation(
                    out_sb[:, d0 * TW:d0 * TW + fw], pse[:, 0:fw], SIG,
                    bias=b3t[:], scale=1.0)

            # d-block 0 tiles, then conv chunks 0..5 interleaved with
            # d-block 1 tiles, then the rest of the conv.
            if emit_rest:
                for it in range(NIT):
                    stage_bc(it, 0)
                for it in range(NIT):
                    stage_bc(it, 1)
                    if trunc != 'bc' and it >= 4 and it % 2 == 0:
                        stage_d(it // 2 - 2)   # dc 0..2 while dt1 runs
                if trunc != 'bc':
                    for dc in range(3, NDCH):
                        stage_d(dc)
                        stage_e(dc - 3)
                    for dc in range(NDCH - 3, NDCH):
                        stage_e(dc)
                else:
                    nc.vector.memset(out_sb[:], 0.0)
                nc.scalar.dma_start(out_d[:], out_sb[:])
    _legalize_waits(nc)
    return nc


_CACHE = {}


def _prep(smp_weight):
    key = hash(smp_weight.tobytes())
    if key in _CACHE:
        return _CACHE[key]
    smp = np.asarray(smp_weight, dtype=np.float32).reshape(T, N, D, T)
    geom = _geometry(smp != 0)
    nc = _build_program(geom)
    core = [_build_core_data(smp, geom, th) for th in range(2)]
    _CACHE[key] = (nc, core)
    return _CACHE[key]


def kernel(feature, smp_weight, w0, b0, w1, b1, w2, b2, w3, b3,
           _trace=False):
    feature = np.asarray(feature, dtype=np.float32)
    nc, core = _prep(np.asarray(smp_weight, dtype=np.float32))

    # w0: [128(c%), N, 2(c-chunk), DIM0]
    w0p = np.ascontiguousarray(
        np.asarray(w0, dtype=np.float32).transpose(2, 1, 0).reshape(
            N, 2, 128, DIM0).transpose(2, 0, 1, 3)).astype(BF)
    w1p = np.asarray(w1, dtype=np.float32).T.reshape(4, 128, DIM1)
    w2p = np.asarray(w2, dtype=np.float32).transpose(2, 3, 1, 0).reshape(
        9, DIM1, DIM1)
    w3p = np.asarray(w3, dtype=np.float32).T                # (128, 1)
    wsm = np.zeros((128, 14 * DIM1 + 1), dtype=np.float32)
    for c in range(4):
        wsm[:, c * DIM1:(c + 1) * DIM1] = w1p[c]
    for j in range(9):
        wsm[:, (4 + j) * DIM1:(5 + j) * DIM1] = w2p[j]
    wsm[:, 13 * DIM1:13 * DIM1 + 1] = w3p
    wsm = wsm.astype(BF)
    b0r = np.asarray(b0, dtype=np.float32).reshape(1, DIM0).astype(BF)
    b123c = []
    for th in range(2):
        a = np.zeros((128, 5), dtype=np.float32)
        a[:, 0] = np.asarray(b1, dtype=np.float32).ravel()
        a[:, 1] = np.asarray(b2, dtype=np.float32).ravel()
        a[0, 2] = float(np.asarray(b3, dtype=np.float32).ravel()[0])
        a[:, 3] = 0.0 if th == 0 else 1.0   # left halo (t=-1 / t=127)
        a[:, 4] = 1.0 if th == 0 else 0.0   # right halo (t=129 / t=256)
        b123c.append(a)

    # feat' per b: [128, 2(c-chunk), 384] covering absolute tau in [-64, 320)
    feats = []
    for b in range(B):
        f = np.zeros((C_IN, AW), dtype=np.float32)
        f[:, -TAU0:-TAU0 + T] = feature[b]
        feats.append(np.ascontiguousarray(
            f.reshape(2, 128, AW).transpose(1, 0, 2)).astype(BF))

    in_maps = []
    for cid in range(8):
        b, th = cid // 2, cid % 2
        idx, wp, cm, _ = core[th]
        in_maps.append({
            "feat": feats[b], "w0t": w0p, "wp": wp, "gidx": idx,
            "wsm": wsm, "b0r": b0r, "b123": b123c[th],
        })
    res = run_bass_kernel_spmd(nc, in_maps, core_ids=list(range(8)),
                               trace=_trace)
    out = np.empty((B, D, T), dtype=np.float32)
    for cid in range(8):
        b, th = cid // 2, cid % 2
        full = res.results[cid]["out"].reshape(D, TW)
        out[b, :, th * 128:(th + 1) * 128] = full[:, 1:TW - 1]
    if _trace:
        return out, res
    return out


# revision 48
# speedup vs baseline: 1.0116x; 1.0116x over previous
"""Trainium2 Bass kernel for nn_BoundaryModule_38422777430159.

Reference computation (B=4, C=256, T=256, N=10, D=40, DIM0=512, DIM1=128):
  x1 = sample(feature)            # (B,C,N,D,T) via (T, N*D*T) smp matmul
  x2 = leaky(einsum('bcndt,ocn->bodt', x1, w0) + b0)
  x3 = leaky(w1 @ x2 + b1)        # 1x1 conv
  x4 = leaky(conv3x3(x3, w2) + b2)
  out = sigmoid(w3 @ x4 + b3)     # (B, D, T)

Device strategy (8 cores SPMD; core = (b = i//2, t-half th = i%2), 1-col halo):

The smp matrix is linear interpolation: each output column (n,d,t) touches
<=2 adjacent tau rows of G_n = w0_n.T @ feature.  The dense contraction
over (n,tau)=2560 rows is therefore ~99% zeros.  This kernel packs, for
each (13 t x 20 d) output tile, exactly the tau-bands it needs into a few
dense 128-row chunks:

  A:    A[n,win] = feat_win.T @ w0_n  (bf16, 3 tau-windows of 128/n)
        -> DMA to a DRAM scratch (3840 rows of [512], +1 row = b0)
  pack: per chunk, one gpsimd indirect-DMA gather (idx per partition,
        host-computed from the smp nonzero pattern); row 0 = b0 row so the
        block0 bias rides the matmul
  B:    per tile: ~2.5 chunks x 4 o-blocks of [128,260] bf16 matmuls
        against host-packed W' slices; Act-engine Prelu -> x2 (bf16)
  C:    x3 = w1 @ x2 (+ b1 x colmask via a rank-1 matmul so halo columns
        stay exactly 0), Prelu written strided into the conv pad buffer
  D/E:  3x3 conv (bf16) + sigmoid(w3 @ x4 + b3) as before.

All matmuls bf16 (1 cyc/row at any free size); PSUM accumulates fp32.
"""
import os
import sys

for _p in ("/opt/trn_rl_repo", "/root/.axon_site/_ro/trn_rl_repo"):
    if os.path.isdir(_p) and _p not in sys.path:
        sys.path.append(_p)

import numpy as np
import ml_dtypes

import concourse.bass as bass
import concourse.tile as tile
from concourse import mybir
from concourse.bass_utils import run_bass_kernel_spmd

T = 256
N = 10
D = 40
B = 4
C_IN = 256
DIM0 = 512
DIM1 = 128

TW = 130            # t-window incl 1-col halo each side
DI, DD = 13, 20     # stage-B tile: 13 t-cols x 20 d-rows
FW = DI * DD        # 260 matmul columns per tile
NIT, NDT = TW // DI, D // DD   # 10 x 2 tiles
KG = 6              # i-tiles grouped per chunk-set
WIN = 3             # tau windows per n in stage A
TAU0 = -64          # window 0 starts at absolute tau = -64
AW = WIN * 128      # 384 feat' columns
NAROW = N * AW      # 3840 real A rows; row 3840 = b0
CAP = 127           # chunk rows 1..127 carry data; row 0 = b0
DCH = 3             # conv d-rows per psum group
NDCH = (D + DCH - 1) // DCH  # 14

F32 = mybir.dt.float32
BF16 = mybir.dt.bfloat16
I32 = mybir.dt.int32
BF = ml_dtypes.bfloat16


def _legalize_waits(nc, limit=1):
    """This walrus build allows a single embedded sync wait per real
    instruction; move the excess onto standalone NoOp wait-carriers."""
    moved = 0
    for f in nc.m.functions:
        for bb in f.blocks:
            il = bb.instructions
            out = []
            changed = False
            for inst in il:
                si = inst.sync_info
                ty = type(inst).__name__
                if (si and si.on_wait and len(si.on_wait) > limit
                        and ty not in ("InstEventSemaphore", "InstNoOp")):
                    keep = si.on_wait[-limit:]
                    for w in si.on_wait[:-limit]:
                        out.append(mybir.InstNoOp(
                            name=f"waitnop-{nc.next_id()}",
                            sync_info=mybir.SyncInfo(on_wait=[w], on_update=[]),
                            bass_nofuse=True,
                            engine=inst.engine,
                        ))
                        moved += 1
                    inst.sync_info = mybir.SyncInfo(
                        on_wait=keep, on_update=si.on_update)
                    changed = True
                out.append(inst)
            if changed:
                bb.instructions = out
    return moved


# ---------------------------------------------------------------------------
# host-side geometry: which (n, tau) rows each stage-B tile needs, grouped
# into shared chunk-sets; identical program structure for every core.
# ---------------------------------------------------------------------------

def _tile_cols(it, th):
    """absolute i for the 13 columns of i-tile `it` on half `th` (may be
    outside [0, T): those columns are pad)."""
    return [it * DI + il - 1 + 128 * th for il in range(DI)]


def _geometry(nzmask):
    """nzmask: [T(tau), N, D, T(i)] bool of the smp matrix.
    Returns the program structure + per-th packing:
      groups: list of dicts with tiles, nch, and per-th
              (chunk_rows[th][c] = list of (n, tau) or None) ...
      tiles:  dict (it,dt) -> (group_idx, s0, s1, b0_slot_local)
    """
    # per (th, it): valid i list and column index
    tile_rows = {}
    for th in range(2):
        for it in range(NIT):
            cols = _tile_cols(it, th)
            valid = [i for i in cols if 0 <= i < T]
            for dt in range(NDT):
                if valid:
                    sub = nzmask[:, :, dt * DD:(dt + 1) * DD, :][:, :, :, valid]
                    tn = np.argwhere(sub.any(axis=(2, 3)))  # (tau, n)
                    rows = set((int(n_), int(t_)) for t_, n_ in tn)
                else:
                    rows = set()
                tile_rows[(th, it, dt)] = rows

    groups = []
    tiles = {}
    chunk_base = 0
    for dt in range(NDT):
        for g0 in range(0, NIT, KG):
            its = list(range(g0, min(g0 + KG, NIT)))
            per_th = []
            spans = [{}, {}]
            for th in range(2):
                sets = [tile_rows[(th, it, dt)] for it in its]
                allrows = sorted(set().union(*sets))
                if allrows:
                    use = {r: [k for k, s in enumerate(sets) if r in s]
                           for r in allrows}
                    allrows.sort(key=lambda r: (float(np.mean(use[r])), r))
                pos = {r: j for j, r in enumerate(allrows)}
                nch = max(1, -(-len(allrows) // CAP))
                chunk_rows = []
                for c in range(nch):
                    chunk_rows.append(allrows[c * CAP:(c + 1) * CAP])
                per_th.append(chunk_rows)
                for it, s in zip(its, sets):
                    if s:
                        ps = [pos[r] for r in s]
                        spans[th][it] = (min(ps) // CAP, max(ps) // CAP)
            nch = max(len(per_th[0]), len(per_th[1]))
            for it in its:
                sp = [spans[th][it] for th in range(2) if it in spans[th]]
                if sp:
                    s0 = min(a for a, b in sp)
                    s1 = max(b for a, b in sp)
                else:
                    s0 = s1 = 0
                tiles[(it, dt)] = (len(groups), s0, s1)
            groups.append(dict(dt=dt, its=its, nch=nch, per_th=per_th,
                               chunk_base=chunk_base))
            chunk_base += nch
    return groups, tiles, chunk_base


def _build_core_data(smp, geom, th):
    """Per-t-half gather indices and packed W' slices (shared across b)."""
    groups, tiles, nch_total = geom
    # smp padded in i: index ip = i+1 in [0, 258)
    smp_pad = np.zeros((T, N, D, T + 2), dtype=np.float32)
    smp_pad[:, :, :, 1:T + 1] = smp

    idx = np.full((128, nch_total), NAROW, dtype=np.int32)  # default: b0 row
    for g in groups:
        rows_c = g["per_th"][th] if th < len(g["per_th"]) else []
        for c in range(g["nch"]):
            rows = rows_c[c] if c < len(rows_c) else []
            for j, (n_, tau) in enumerate(rows):
                idx[1 + j, g["chunk_base"] + c] = n_ * AW + (tau - TAU0)

    # slot list in program emission order: for each tile (dt-major), its
    # slots s0..s1
    slots = []
    for dt in range(NDT):
        for it in range(NIT):
            gi, s0, s1 = tiles[(it, dt)]
            for s in range(s0, s1 + 1):
                slots.append((it, dt, gi, s, s == s0))
    nslot = len(slots)

    wp = np.zeros((nslot, 128, FW), dtype=BF)
    ip_cols = {}
    for it in range(NIT):
        ip_cols[it] = np.array([min(max(i + 1, 0), T + 1)
                                for i in _tile_cols(it, th)])
    for si, (it, dt, gi, s, isfirst) in enumerate(slots):
        g = groups[gi]
        rows_c = g["per_th"][th]
        rows = rows_c[s] if s < len(rows_c) else []
        dsl = slice(dt * DD, (dt + 1) * DD)
        if rows:
            ns = np.array([r[0] for r in rows])
            taus = np.array([r[1] for r in rows])
            vals = smp_pad[taus, ns][:, dsl, :][:, :, ip_cols[it]]
            wp[si, 1:1 + len(rows)] = vals.reshape(len(rows), FW).astype(BF)
        if isfirst:
            mask = np.array([1.0 if 0 <= i < T else 0.0
                             for i in _tile_cols(it, th)], dtype=np.float32)
            wp[si, 0] = np.tile(mask, DD).astype(BF)

    # colmask for the b1 rank-1 matmul, tiled per i-tile: [1, NIT*FW]
    cm = np.zeros((1, NIT * FW), dtype=BF)
    for it in range(NIT):
        mask = np.array([1.0 if 0 <= i < T else 0.0
                         for i in _tile_cols(it, th)], dtype=np.float32)
        cm[0, it * FW:(it + 1) * FW] = np.tile(mask, DD).astype(BF)
    return idx, wp, cm, slots


# ---------------------------------------------------------------------------
# program
# ---------------------------------------------------------------------------

def _build_program(geom, trunc=None):
    # trunc: debug levels 'a' (stage A only), 'pack', 'bc' (no conv)
    groups, tiles, nch_total = geom
    slots = []
    for dt in range(NDT):
        for it in range(NIT):
            gi, s0, s1 = tiles[(it, dt)]
            for s in range(s0, s1 + 1):
                slots.append((it, dt, gi, s))
    nslot = len(slots)
    WMAX = max(s1 - s0 + 1 for _, s0, s1 in tiles.values())

    nc = bass.Bass(trn_type="TRN2", num_swdge_queues=4)
    PRELU = mybir.ActivationFunctionType.Prelu
    SIG = mybir.ActivationFunctionType.Sigmoid

    feat_d = nc.dram_tensor("feat", [128, 2, AW], BF16, kind="ExternalInput")
    w0_d = nc.dram_tensor("w0t", [128, N, 2, DIM0], BF16, kind="ExternalInput")
    wp_d = nc.dram_tensor("wp", [nslot, 128, FW], BF16, kind="ExternalInput")
    idx_d = nc.dram_tensor("gidx", [128, nch_total], I32, kind="ExternalInput")
    # w1 (4x128) | w2 (9x128) | w3 (1 col) along the free dim
    wsm_d = nc.dram_tensor("wsm", [128, 14 * DIM1 + 1], BF16,
                           kind="ExternalInput")
    b0r_d = nc.dram_tensor("b0r", [1, DIM0], BF16, kind="ExternalInput")
    # per-partition columns: b1 | b2 | b3(row0) | halo maskL | maskR
    b123_d = nc.dram_tensor("b123", [128, 5], F32, kind="ExternalInput")
    a_d = nc.dram_tensor("adram", [NAROW + 1, DIM0], BF16, kind="Internal")
    out_d = nc.dram_tensor("out", [1, D * TW], F32, kind="ExternalOutput")

    with tile.TileContext(nc) as tc:
        with (
            tc.tile_pool(name="inp", bufs=1) as inp,
            tc.tile_pool(name="asb", bufs=4) as asb,
            tc.tile_pool(name="apk", bufs=1) as apk,
            tc.tile_pool(name="wst", bufs=4) as wst,
            tc.tile_pool(name="x2p", bufs=2) as x2p,
            tc.tile_pool(name="x3p", bufs=1) as x3p,
            tc.tile_pool(name="x4p", bufs=2) as x4p,
            tc.tile_pool(name="outp", bufs=1) as outp,
            tc.tile_pool(name="psb", bufs=1, space="PSUM") as psb,
            tc.tile_pool(name="psg", bufs=2, space="PSUM") as psg,
        ):
            # ---- input DMAs (few, large; feat + first w0 group first) ----
            ft = inp.tile([128, 2, AW], BF16, tag="f", name="feat_sb")
            nc.sync.dma_start(ft[:], feat_d[:])
            feat = [ft[:, c] for c in range(2)]
            w0sb = inp.tile([128, N, 2, DIM0], BF16, tag="w0", name="w0_sb")
            NG = [(0, 3), (3, 6), (6, 9), (9, 10)]
            for gi_, (n0, n1) in enumerate(NG):
                eng = (nc.sync, nc.scalar)[gi_ % 2]
                eng.dma_start(w0sb[:, n0:n1], w0_d[:, n0:n1])
            w0t = [[w0sb[:, n, c] for c in range(2)] for n in range(N)]
            b0r_t = inp.tile([1, DIM0], BF16, tag="b0r", name="b0r_sb")
            nc.scalar.dma_start(b0r_t[:], b0r_d[:])
            # b0 row of the A scratch
            nc.scalar.dma_start(a_d[NAROW:NAROW + 1, :], b0r_t[:])
            idx_t = inp.tile([128, nch_total], I32, tag="idx", name="gidx_sb")
            nc.sync.dma_start(idx_t[:], idx_d[:])
            wsm = inp.tile([128, 14 * DIM1 + 1], BF16, tag="wsm", name="wsm_sb")
            nc.scalar.dma_start(wsm[:], wsm_d[:])
            w1t = [wsm[:, c * DIM1:(c + 1) * DIM1] for c in range(4)]
            w2t = [wsm[:, (4 + j) * DIM1:(5 + j) * DIM1] for j in range(9)]
            w3t = wsm[:, 13 * DIM1:13 * DIM1 + 1]
            b123 = inp.tile([128, 5], F32, tag="b123", name="b123_sb")
            nc.sync.dma_start(b123[:], b123_d[:])
            b1t = b123[:, 0:1]
            b2t = b123[:, 1:2]
            b3t = b123[0:1, 2:3]
            mlr_t = inp.tile([128, 2], BF16, tag="mlr", name="mlr_sb")
            nc.vector.tensor_copy(mlr_t[:], b123[:, 3:5])

            # ---- PE warm-up: keep the PE busy (and its p-state ramped)
            # while the feat/w0 DMAs land and later while the gathers run.
            warm = inp.tile([128, DIM0], BF16, tag="wm", name="warm_sb")
            nc.vector.memset(warm[:].bitcast(mybir.dt.uint16), 0)
            wps = psb.tile([1, DIM0], F32, tag="b3", name="warm_ps")

            def warm_mm(k, free=64):
                nc.tensor.matmul(wps[:, 0:free], warm[:, 0:1], warm[:, 0:free],
                                 start=True, stop=True)

            for k in range(10):
                warm_mm(k)

            # ---- stage A: A[n,win] = feat_win.T @ w0_n -> adram ----
            for n in range(N):
                a_n = asb.tile([128, WIN, DIM0], BF16, tag=f"a{n % 4}",
                               name=f"a{n}")
                for win in range(WIN):
                    k = n * WIN + win
                    ps = psb.tile([128, DIM0], F32, tag=f"b{k % 4}",
                                  name=f"psa{n}_{win}")
                    for c in range(2):
                        nc.tensor.matmul(
                            ps[:],
                            feat[c][:, win * 128:(win + 1) * 128],
                            w0t[n][c][:],
                            start=(c == 0), stop=(c == 1),
                        )
                    if (n * WIN + win) % 2:
                        nc.vector.tensor_copy(a_n[:, win], ps[:])
                    else:
                        nc.scalar.activation(
                            a_n[:, win], ps[:],
                            mybir.ActivationFunctionType.Copy,
                            bias=0.0, scale=1.0)
                nc.sync.dma_start(
                    a_d[n * AW:(n + 1) * AW, :].rearrange(
                        "(w p) e -> p w e", p=128),
                    a_n[:])

            # ---- pack: one indirect gather per chunk ----
            apack = []
            if trunc != 'a':
                for ch in range(nch_total):
                    g = apk.tile([128, DIM0], BF16, tag=f"ap{ch}", name=f"ap{ch}")
                    h = nc.gpsimd.indirect_dma_start(
                        out=g[:], out_offset=None, in_=a_d[:],
                        in_offset=bass.IndirectOffsetOnAxis(
                            ap=idx_t[:, ch:ch + 1], axis=0))
                    q = ch % 4
                    if q:
                        h.ins.queue = f"qPoolDynamic{q}"
                    apack.append(g)

            if trunc in ('a', 'pack'):
                out_sb = outp.tile([1, D * TW], F32, tag="os", name="out_sb")
                nc.vector.memset(out_sb[:], 0.0)
                for ch in range(len(apack)):
                    nc.vector.tensor_copy(out_sb[:, ch:ch + 1],
                                          apack[ch][0:1, 0:1].bitcast(BF16)[:, 0:1] if False
                                          else apack[ch][0:1, 0:1])
                nc.scalar.dma_start(out_d[:], out_sb[:])

            # ---- stages B+C per tile; conv chunks interleaved per d-block --
            emit_rest = trunc not in ('a', 'pack')
            pad = x3p.tile([128, D + 2, TW + 2], BF16, tag="pad", name="padbuf")
            if emit_rest:
                nc.vector.memset(pad[:].bitcast(mybir.dt.uint16), 0)
                out_sb = outp.tile([1, D * TW], F32, tag="os", name="out_sb")
            x4cs = [None] * NDCH

            def stage_bc(it, dt):
                gi, s0, s1 = tiles[(it, dt)]
                g = groups[gi]
                ns = s1 - s0 + 1
                si0 = slot_id[(it, dt, s0)]
                wt = wst.tile([128, WMAX, FW], BF16, tag="w",
                              name=f"wp{it}_{dt}")
                eng = (nc.sync, nc.scalar)[(it + dt) % 2]
                eng.dma_start(wt[:, 0:ns],
                              wp_d[si0:si0 + ns].transpose((1, 0, 2)))
                acc = [psb.tile([128, FW], F32, tag=f"b{ob}",
                                name=f"psb{it}_{dt}_{ob}") for ob in range(4)]
                for s in range(s0, s1 + 1):
                    ch = g["chunk_base"] + s
                    for ob in range(4):
                        nc.tensor.matmul(
                            acc[ob][:],
                            apack[ch][:, ob * 128:(ob + 1) * 128],
                            wt[:, s - s0],
                            start=(s == s0), stop=(s == s1),
                        )
                x2c = []
                for ob in range(4):
                    yt = x2p.tile([128, FW], BF16, tag=f"x2_{ob}",
                                  name=f"x2_{it}_{dt}_{ob}")
                    if ob < 2:
                        nc.scalar.activation(yt[:], acc[ob][:], PRELU,
                                             bias=0.0, scale=1.0, alpha=0.01)
                    else:
                        nc.vector.tensor_copy(yt[:], acc[ob][:])
                        nc.vector.scalar_tensor_tensor(
                            yt[:], yt[:], 0.01, yt[:],
                            mybir.AluOpType.mult, mybir.AluOpType.max)
                    x2c.append(yt)
                psc = psg.tile([128, FW], F32, tag="g", name=f"psc{it}_{dt}")
                for ob in range(4):
                    nc.tensor.matmul(psc[:], w1t[ob][:], x2c[ob][:],
                                     start=(ob == 0), stop=(ob == 3))
                nc.scalar.activation(
                    pad[:, 1 + dt * DD:1 + (dt + 1) * DD,
                        1 + it * DI:1 + (it + 1) * DI],
                    psc[:].rearrange("p (d t) -> p d t", d=DD),
                    PRELU, bias=b1t, scale=1.0, alpha=0.01)
                # exact zero-padding: the per-core invalid halo column
                # (t = -1 for th0, t = 256 for th1) is scaled by a 0/1 mask
                if it == 0:
                    nc.vector.tensor_tensor(
                        pad[:, 1 + dt * DD:1 + (dt + 1) * DD, 1],
                        pad[:, 1 + dt * DD:1 + (dt + 1) * DD, 1],
                        mlr_t[:, 0:1].to_broadcast([128, DD]),
                        mybir.AluOpType.mult)
                if it == NIT - 1:
                    nc.vector.tensor_tensor(
                        pad[:, 1 + dt * DD:1 + (dt + 1) * DD, TW],
                        pad[:, 1 + dt * DD:1 + (dt + 1) * DD, TW],
                        mlr_t[:, 1:2].to_broadcast([128, DD]),
                        mybir.AluOpType.mult)

            slot_id = {}
            k = 0
            for dt in range(NDT):
                for it in range(NIT):
                    gi, s0, s1 = tiles[(it, dt)]
                    for s in range(s0, s1 + 1):
                        slot_id[(it, dt, s)] = k
                        k += 1

            def stage_d(dc):
                d0 = dc * DCH
                nd = min(DCH, D - d0)
                fw = nd * TW
                psd = psg.tile([128, DCH * TW], F32, tag="d", name=f"psd{dc}")
                for j in range(9):
                    dy, dx = j // 3, j % 3
                    nc.tensor.matmul(
                        psd[:, 0:fw],
                        w2t[j][:],
                        pad[:, d0 + dy:d0 + dy + nd, dx:dx + TW],
                        start=(j == 0), stop=(j == 8),
                    )
                x4c = x4p.tile([128, DCH * TW], BF16, tag=f"x4_{dc}",
                               name=f"x4_{dc}")
                nc.scalar.activation(x4c[:, 0:fw], psd[:, 0:fw], PRELU,
                                     bias=b2t[:], scale=1.0, alpha=0.01)
                x4cs[dc] = x4c

            def stage_e(dc):
                d0 = dc * DCH
                fw = min(DCH, D - d0) * TW
                pse = psg.tile([1, DCH * TW], F32, tag="g", name=f"pse{dc}")
                nc.tensor.matmul(pse[:, 0:fw], w3t[:], x4cs[dc][:, 0:fw],
                                 start=True, stop=True)
                nc.scalar.activation(
                    out_sb[:, d0 * TW:d0 * TW + fw], pse[:, 0:fw], SIG,
                    bias=b3t[:], scale=1.0)

            # d-block 0 tiles, then conv chunks 0..5 interleaved with
            # d-block 1 tiles, then the rest of the conv.
            if emit_rest:
                for it in range(NIT):
                    stage_bc(it, 0)
                for it in range(NIT):
                    stage_bc(it, 1)
                    if trunc != 'bc' and it >= 4 and it % 2 == 0:
                        stage_d(it // 2 - 2)   # dc 0..2 while dt1 runs
                if trunc != 'bc':
                    for dc in range(3, NDCH):
                        stage_d(dc)
                        stage_e(dc - 3)
                    for dc in range(NDCH - 3, NDCH):
                        stage_e(dc)
                else:
                    nc.vector.memset(out_sb[:], 0.0)
                nc.scalar.dma_start(out_d[:], out_sb[:])
    _legalize_waits(nc)
    return nc


_CACHE = {}


def _prep(smp_weight):
    key = hash(smp_weight.tobytes())
    if key in _CACHE:
        return _CACHE[key]
    smp = np.asarray(smp_weight, dtype=np.float32).reshape(T, N, D, T)
    geom = _geometry(smp != 0)
    nc = _build_program(geom)
    core = [_build_core_data(smp, geom, th) for th in range(2)]
    _CACHE[key] = (nc, core)
    return _CACHE[key]


def kernel(feature, smp_weight, w0, b0, w1, b1, w2, b2, w3, b3,
           _trace=False):
    feature = np.asarray(feature, dtype=np.float32)
    nc, core = _prep(np.asarray(smp_weight, dtype=np.float32))

    # w0: [128(c%), N, 2(c-chunk), DIM0]
    w0p = np.ascontiguousarray(
        np.asarray(w0, dtype=np.float32).transpose(2, 1, 0).reshape(
            N, 2, 128, DIM0).transpose(2, 0, 1, 3)).astype(BF)
    w1p = np.asarray(w1, dtype=np.float32).T.reshape(4, 128, DIM1)
    w2p = np.asarray(w2, dtype=np.float32).transpose(2, 3, 1, 0).reshape(
        9, DIM1, DIM1)
    w3p = np.asarray(w3, dtype=np.float32).T                # (128, 1)
    wsm = np.zeros((128, 14 * DIM1 + 1), dtype=np.float32)
    for c in range(4):
        wsm[:, c * DIM1:(c + 1) * DIM1] = w1p[c]
    for j in range(9):
        wsm[:, (4 + j) * DIM1:(5 + j) * DIM1] = w2p[j]
    wsm[:, 13 * DIM1:13 * DIM1 + 1] = w3p
    wsm = wsm.astype(BF)
    b0r = np.asarray(b0, dtype=np.float32).reshape(1, DIM0).astype(BF)
    b123c = []
    for th in range(2):
        a = np.zeros((128, 5), dtype=np.float32)
        a[:, 0] = np.asarray(b1, dtype=np.float32).ravel()
        a[:, 1] = np.asarray(b2, dtype=np.float32).ravel()
        a[0, 2] = float(np.asarray(b3, dtype=np.float32).ravel()[0])
        a[:, 3] = 0.0 if th == 0 else 1.0   # left halo (t=-1 / t=127)
        a[:, 4] = 1.0 if th == 0 else 0.0   # right halo (t=129 / t=256)
        b123c.append(a)

    # feat' per b: [128, 2(c-chunk), 384] covering absolute tau in [-64, 320)
    feats = []
    for b in range(B):
        f = np.zeros((C_IN, AW), dtype=np.float32)
        f[:, -TAU0:-TAU0 + T] = feature[b]
        feats.append(np.ascontiguousarray(
            f.reshape(2, 128, AW).transpose(1, 0, 2)).astype(BF))

    in_maps = []
    for cid in range(8):
        b, th = cid // 2, cid % 2
        idx, wp, cm, _ = core[th]
        in_maps.append({
            "feat": feats[b], "w0t": w0p, "wp": wp, "gidx": idx,
            "wsm": wsm, "b0r": b0r, "b123": b123c[th],
        })
    res = run_bass_kernel_spmd(nc, in_maps, core_ids=list(range(8)),
                               trace=_trace)
    out = np.empty((B, D, T), dtype=np.float32)
    for cid in range(8):
        b, th = cid // 2, cid % 2
        full = res.results[cid]["out"].reshape(D, TW)
        out[b, :, th * 128:(th + 1) * 128] = full[:, 1:TW - 1]
    if _trace:
        return out, res
    return out


# revision 49
# speedup vs baseline: 1.0770x; 1.0646x over previous
"""Trainium2 Bass kernel for nn_BoundaryModule_38422777430159.

Reference computation (B=4, C=256, T=256, N=10, D=40, DIM0=512, DIM1=128):
  x1 = sample(feature)            # (B,C,N,D,T) via (T, N*D*T) smp matmul
  x2 = leaky(einsum('bcndt,ocn->bodt', x1, w0) + b0)
  x3 = leaky(w1 @ x2 + b1)        # 1x1 conv
  x4 = leaky(conv3x3(x3, w2) + b2)
  out = sigmoid(w3 @ x4 + b3)     # (B, D, T)

Device strategy (8 cores SPMD; core = (b = i//2, t-half th = i%2), 1-col halo):

The smp matrix is linear interpolation: each output column (n,d,t) touches
<=2 adjacent tau rows of G_n = w0_n.T @ feature.  The dense contraction
over (n,tau)=2560 rows is therefore ~99% zeros.  This kernel packs, for
each (13 t x 20 d) output tile, exactly the tau-bands it needs into a few
dense 128-row chunks:

  A:    A[n,win] = feat_win.T @ w0_n  (bf16, 3 tau-windows of 128/n)
        -> DMA to a DRAM scratch (3840 rows of [512], +1 row = b0)
  pack: per chunk, one gpsimd indirect-DMA gather (idx per partition,
        host-computed from the smp nonzero pattern); row 0 = b0 row so the
        block0 bias rides the matmul
  B:    per tile: ~2.5 chunks x 4 o-blocks of [128,260] bf16 matmuls
        against host-packed W' slices; Act-engine Prelu -> x2 (bf16)
  C:    x3 = w1 @ x2 (+ b1 x colmask via a rank-1 matmul so halo columns
        stay exactly 0), Prelu written strided into the conv pad buffer
  D/E:  3x3 conv (bf16) + sigmoid(w3 @ x4 + b3) as before.

All matmuls bf16 (1 cyc/row at any free size); PSUM accumulates fp32.
"""
import os
import sys

for _p in ("/opt/trn_rl_repo", "/root/.axon_site/_ro/trn_rl_repo"):
    if os.path.isdir(_p) and _p not in sys.path:
        sys.path.append(_p)

import numpy as np
import ml_dtypes

import concourse.bass as bass
import concourse.tile as tile
from concourse import mybir
from concourse.bass_utils import run_bass_kernel_spmd

T = 256
N = 10
D = 40
B = 4
C_IN = 256
DIM0 = 512
DIM1 = 128

TW = 130            # t-window incl 1-col halo each side
DI, DD = 13, 20     # stage-B tile: 13 t-cols x 20 d-rows
FW = DI * DD        # 260 matmul columns per tile
NIT, NDT = TW // DI, D // DD   # 10 x 2 tiles
KG = 3              # i-tiles grouped per chunk-set
WIN = 3             # tau windows per n in stage A
TAU0 = -64          # window 0 starts at absolute tau = -64
AW = WIN * 128      # 384 feat' columns
NAROW = N * AW      # 3840 real A rows; row 3840 = b0
CAP = 127           # chunk rows 1..127 carry data; row 0 = b0
DCH = 3             # conv d-rows per psum group
NDCH = (D + DCH - 1) // DCH  # 14

F32 = mybir.dt.float32
BF16 = mybir.dt.bfloat16
I32 = mybir.dt.int32
BF = ml_dtypes.bfloat16


def _legalize_waits(nc, limit=1):
    """This walrus build allows a single embedded sync wait per real
    instruction; move the excess onto standalone NoOp wait-carriers."""
    moved = 0
    for f in nc.m.functions:
        for bb in f.blocks:
            il = bb.instructions
            out = []
            changed = False
            for inst in il:
                si = inst.sync_info
                ty = type(inst).__name__
                if (si and si.on_wait and len(si.on_wait) > limit
                        and ty not in ("InstEventSemaphore", "InstNoOp")):
                    keep = si.on_wait[-limit:]
                    for w in si.on_wait[:-limit]:
                        out.append(mybir.InstNoOp(
                            name=f"waitnop-{nc.next_id()}",
                            sync_info=mybir.SyncInfo(on_wait=[w], on_update=[]),
                            bass_nofuse=True,
                            engine=inst.engine,
                        ))
                        moved += 1
                    inst.sync_info = mybir.SyncInfo(
                        on_wait=keep, on_update=si.on_update)
                    changed = True
                out.append(inst)
            if changed:
                bb.instructions = out
    return moved


# ---------------------------------------------------------------------------
# host-side geometry: which (n, tau) rows each stage-B tile needs, grouped
# into shared chunk-sets; identical program structure for every core.
# ---------------------------------------------------------------------------

def _tile_cols(it, th):
    """absolute i for the 13 columns of i-tile `it` on half `th` (may be
    outside [0, T): those columns are pad)."""
    return [it * DI + il - 1 + 128 * th for il in range(DI)]


def _geometry(nzmask):
    """nzmask: [T(tau), N, D, T(i)] bool of the smp matrix.
    Returns the program structure + per-th packing:
      groups: list of dicts with tiles, nch, and per-th
              (chunk_rows[th][c] = list of (n, tau) or None) ...
      tiles:  dict (it,dt) -> (group_idx, s0, s1, b0_slot_local)
    """
    # per (th, it): valid i list and column index
    tile_rows = {}
    for th in range(2):
        for it in range(NIT):
            cols = _tile_cols(it, th)
            valid = [i for i in cols if 0 <= i < T]
            for dt in range(NDT):
                if valid:
                    sub = nzmask[:, :, dt * DD:(dt + 1) * DD, :][:, :, :, valid]
                    tn = np.argwhere(sub.any(axis=(2, 3)))  # (tau, n)
                    rows = set((int(n_), int(t_)) for t_, n_ in tn)
                else:
                    rows = set()
                tile_rows[(th, it, dt)] = rows

    groups = []
    tiles = {}
    chunk_base = 0
    for dt in range(NDT):
        for g0 in range(0, NIT, KG):
            its = list(range(g0, min(g0 + KG, NIT)))
            per_th = []
            spans = [{}, {}]
            for th in range(2):
                sets = [tile_rows[(th, it, dt)] for it in its]
                allrows = sorted(set().union(*sets))
                if allrows:
                    use = {r: [k for k, s in enumerate(sets) if r in s]
                           for r in allrows}
                    allrows.sort(key=lambda r: (float(np.mean(use[r])), r))
                pos = {r: j for j, r in enumerate(allrows)}
                nch = max(1, -(-len(allrows) // CAP))
                chunk_rows = []
                for c in range(nch):
                    chunk_rows.append(allrows[c * CAP:(c + 1) * CAP])
                per_th.append(chunk_rows)
                for it, s in zip(its, sets):
                    if s:
                        ps = [pos[r] for r in s]
                        spans[th][it] = (min(ps) // CAP, max(ps) // CAP)
            nch = max(len(per_th[0]), len(per_th[1]))
            for it in its:
                sp = [spans[th][it] for th in range(2) if it in spans[th]]
                if sp:
                    s0 = min(a for a, b in sp)
                    s1 = max(b for a, b in sp)
                else:
                    s0 = s1 = 0
                tiles[(it, dt)] = (len(groups), s0, s1)
            groups.append(dict(dt=dt, its=its, nch=nch, per_th=per_th,
                               chunk_base=chunk_base))
            chunk_base += nch
    return groups, tiles, chunk_base


def _build_core_data(smp, geom, th):
    """Per-t-half gather indices and packed W' slices (shared across b)."""
    groups, tiles, nch_total = geom
    # smp padded in i: index ip = i+1 in [0, 258)
    smp_pad = np.zeros((T, N, D, T + 2), dtype=np.float32)
    smp_pad[:, :, :, 1:T + 1] = smp

    idx = np.full((128, nch_total), NAROW, dtype=np.int32)  # default: b0 row
    for g in groups:
        rows_c = g["per_th"][th] if th < len(g["per_th"]) else []
        for c in range(g["nch"]):
            rows = rows_c[c] if c < len(rows_c) else []
            for j, (n_, tau) in enumerate(rows):
                idx[1 + j, g["chunk_base"] + c] = n_ * AW + (tau - TAU0)

    # slot list in program emission order: for each tile (dt-major), its
    # slots s0..s1
    slots = []
    for dt in range(NDT):
        for it in range(NIT):
            gi, s0, s1 = tiles[(it, dt)]
            for s in range(s0, s1 + 1):
                slots.append((it, dt, gi, s, s == s0))
    nslot = len(slots)

    wp = np.zeros((nslot, 128, FW), dtype=BF)
    ip_cols = {}
    for it in range(NIT):
        ip_cols[it] = np.array([min(max(i + 1, 0), T + 1)
                                for i in _tile_cols(it, th)])
    for si, (it, dt, gi, s, isfirst) in enumerate(slots):
        g = groups[gi]
        rows_c = g["per_th"][th]
        rows = rows_c[s] if s < len(rows_c) else []
        dsl = slice(dt * DD, (dt + 1) * DD)
        if rows:
            ns = np.array([r[0] for r in rows])
            taus = np.array([r[1] for r in rows])
            vals = smp_pad[taus, ns][:, dsl, :][:, :, ip_cols[it]]
            wp[si, 1:1 + len(rows)] = vals.reshape(len(rows), FW).astype(BF)
        if isfirst:
            mask = np.array([1.0 if 0 <= i < T else 0.0
                             for i in _tile_cols(it, th)], dtype=np.float32)
            wp[si, 0] = np.tile(mask, DD).astype(BF)

    # colmask for the b1 rank-1 matmul, tiled per i-tile: [1, NIT*FW]
    cm = np.zeros((1, NIT * FW), dtype=BF)
    for it in range(NIT):
        mask = np.array([1.0 if 0 <= i < T else 0.0
                         for i in _tile_cols(it, th)], dtype=np.float32)
        cm[0, it * FW:(it + 1) * FW] = np.tile(mask, DD).astype(BF)
    return idx, wp, cm, slots


# ---------------------------------------------------------------------------
# program
# ---------------------------------------------------------------------------

def _build_program(geom, trunc=None):
    # trunc: debug levels 'a' (stage A only), 'pack', 'bc' (no conv)
    groups, tiles, nch_total = geom
    slots = []
    for dt in range(NDT):
        for it in range(NIT):
            gi, s0, s1 = tiles[(it, dt)]
            for s in range(s0, s1 + 1):
                slots.append((it, dt, gi, s))
    nslot = len(slots)
    WMAX = max(s1 - s0 + 1 for _, s0, s1 in tiles.values())

    nc = bass.Bass(trn_type="TRN2", num_swdge_queues=4)
    PRELU = mybir.ActivationFunctionType.Prelu
    SIG = mybir.ActivationFunctionType.Sigmoid

    feat_d = nc.dram_tensor("feat", [128, 2, AW], BF16, kind="ExternalInput")
    w0_d = nc.dram_tensor("w0t", [128, N, 2, DIM0], BF16, kind="ExternalInput")
    wp_d = nc.dram_tensor("wp", [nslot, 128, FW], BF16, kind="ExternalInput")
    idx_d = nc.dram_tensor("gidx", [128, nch_total], I32, kind="ExternalInput")
    # w1 (4x128) | w2 (9x128) | w3 (1 col) along the free dim
    wsm_d = nc.dram_tensor("wsm", [128, 14 * DIM1 + 1], BF16,
                           kind="ExternalInput")
    b0r_d = nc.dram_tensor("b0r", [1, DIM0], BF16, kind="ExternalInput")
    # per-partition columns: b1 | b2 | b3(row0) | halo maskL | maskR
    b123_d = nc.dram_tensor("b123", [128, 5], F32, kind="ExternalInput")
    a_d = nc.dram_tensor("adram", [NAROW + 1, DIM0], BF16, kind="Internal")
    out_d = nc.dram_tensor("out", [1, D * TW], F32, kind="ExternalOutput")

    with tile.TileContext(nc) as tc:
        with (
            tc.tile_pool(name="inp", bufs=1) as inp,
            tc.tile_pool(name="asb", bufs=4) as asb,
            tc.tile_pool(name="apk", bufs=1) as apk,
            tc.tile_pool(name="wst", bufs=4) as wst,
            tc.tile_pool(name="x2p", bufs=2) as x2p,
            tc.tile_pool(name="x3p", bufs=1) as x3p,
            tc.tile_pool(name="x4p", bufs=2) as x4p,
            tc.tile_pool(name="outp", bufs=1) as outp,
            tc.tile_pool(name="psb", bufs=1, space="PSUM") as psb,
            tc.tile_pool(name="psg", bufs=2, space="PSUM") as psg,
        ):
            # ---- input DMAs (few, large; feat + first w0 group first) ----
            ft = inp.tile([128, 2, AW], BF16, tag="f", name="feat_sb")
            nc.sync.dma_start(ft[:], feat_d[:])
            feat = [ft[:, c] for c in range(2)]
            w0sb = inp.tile([128, N, 2, DIM0], BF16, tag="w0", name="w0_sb")
            NG = [(0, 3), (3, 6), (6, 9), (9, 10)]
            for gi_, (n0, n1) in enumerate(NG):
                eng = (nc.sync, nc.scalar)[gi_ % 2]
                eng.dma_start(w0sb[:, n0:n1], w0_d[:, n0:n1])
            w0t = [[w0sb[:, n, c] for c in range(2)] for n in range(N)]
            b0r_t = inp.tile([1, DIM0], BF16, tag="b0r", name="b0r_sb")
            nc.scalar.dma_start(b0r_t[:], b0r_d[:])
            # b0 row of the A scratch
            nc.scalar.dma_start(a_d[NAROW:NAROW + 1, :], b0r_t[:])
            idx_t = inp.tile([128, nch_total], I32, tag="idx", name="gidx_sb")
            nc.sync.dma_start(idx_t[:], idx_d[:])
            wsm = inp.tile([128, 14 * DIM1 + 1], BF16, tag="wsm", name="wsm_sb")
            nc.scalar.dma_start(wsm[:], wsm_d[:])
            w1t = [wsm[:, c * DIM1:(c + 1) * DIM1] for c in range(4)]
            w2t = [wsm[:, (4 + j) * DIM1:(5 + j) * DIM1] for j in range(9)]
            w3t = wsm[:, 13 * DIM1:13 * DIM1 + 1]
            b123 = inp.tile([128, 5], F32, tag="b123", name="b123_sb")
            nc.sync.dma_start(b123[:], b123_d[:])
            b1t = b123[:, 0:1]
            b2t = b123[:, 1:2]
            b3t = b123[0:1, 2:3]
            mlr_t = inp.tile([128, 2], BF16, tag="mlr", name="mlr_sb")
            nc.vector.tensor_copy(mlr_t[:], b123[:, 3:5])

            # ---- PE warm-up: keep the PE busy (and its p-state ramped)
            # while the feat/w0 DMAs land and later while the gathers run.
            warm = inp.tile([128, DIM0], BF16, tag="wm", name="warm_sb")
            nc.vector.memset(warm[:].bitcast(mybir.dt.uint16), 0)
            wps = psb.tile([1, DIM0], F32, tag="b3", name="warm_ps")

            def warm_mm(k, free=64):
                nc.tensor.matmul(wps[:, 0:free], warm[:, 0:1], warm[:, 0:free],
                                 start=True, stop=True)

            for k in range(10):
                warm_mm(k)

            # ---- stage A: A[n,win] = feat_win.T @ w0_n -> adram ----
            for n in range(N):
                a_n = asb.tile([128, WIN, DIM0], BF16, tag=f"a{n % 4}",
                               name=f"a{n}")
                for win in range(WIN):
                    k = n * WIN + win
                    ps = psb.tile([128, DIM0], F32, tag=f"b{k % 4}",
                                  name=f"psa{n}_{win}")
                    for c in range(2):
                        nc.tensor.matmul(
                            ps[:],
                            feat[c][:, win * 128:(win + 1) * 128],
                            w0t[n][c][:],
                            start=(c == 0), stop=(c == 1),
                        )
                    if (n * WIN + win) % 2:
                        nc.vector.tensor_copy(a_n[:, win], ps[:])
                    else:
                        nc.scalar.activation(
                            a_n[:, win], ps[:],
                            mybir.ActivationFunctionType.Copy,
                            bias=0.0, scale=1.0)
                nc.sync.dma_start(
                    a_d[n * AW:(n + 1) * AW, :].rearrange(
                        "(w p) e -> p w e", p=128),
                    a_n[:])

            # ---- pack: one indirect gather per chunk ----
            apack = []
            if trunc != 'a':
                for ch in range(nch_total):
                    g = apk.tile([128, DIM0], BF16, tag=f"ap{ch}", name=f"ap{ch}")
                    h = nc.gpsimd.indirect_dma_start(
                        out=g[:], out_offset=None, in_=a_d[:],
                        in_offset=bass.IndirectOffsetOnAxis(
                            ap=idx_t[:, ch:ch + 1], axis=0))
                    q = ch % 4
                    if q:
                        h.ins.queue = f"qPoolDynamic{q}"
                    apack.append(g)

            if trunc in ('a', 'pack'):
                out_sb = outp.tile([1, D * TW], F32, tag="os", name="out_sb")
                nc.vector.memset(out_sb[:], 0.0)
                for ch in range(len(apack)):
                    nc.vector.tensor_copy(out_sb[:, ch:ch + 1],
                                          apack[ch][0:1, 0:1].bitcast(BF16)[:, 0:1] if False
                                          else apack[ch][0:1, 0:1])
                nc.scalar.dma_start(out_d[:], out_sb[:])

            # ---- stages B+C per tile; conv chunks interleaved per d-block --
            emit_rest = trunc not in ('a', 'pack')
            pad = x3p.tile([128, D + 2, TW + 2], BF16, tag="pad", name="padbuf")
            if emit_rest:
                nc.vector.memset(pad[:].bitcast(mybir.dt.uint16), 0)
                out_sb = outp.tile([1, D * TW], F32, tag="os", name="out_sb")
            x4cs = [None] * NDCH

            def stage_bc(it, dt):
                gi, s0, s1 = tiles[(it, dt)]
                g = groups[gi]
                ns = s1 - s0 + 1
                si0 = slot_id[(it, dt, s0)]
                wt = wst.tile([128, WMAX, FW], BF16, tag="w",
                              name=f"wp{it}_{dt}")
                eng = (nc.sync, nc.scalar)[(it + dt) % 2]
                eng.dma_start(wt[:, 0:ns],
                              wp_d[si0:si0 + ns].transpose((1, 0, 2)))
                acc = [psb.tile([128, FW], F32, tag=f"b{ob}",
                                name=f"psb{it}_{dt}_{ob}") for ob in range(4)]
                for s in range(s0, s1 + 1):
                    ch = g["chunk_base"] + s
                    for ob in range(4):
                        nc.tensor.matmul(
                            acc[ob][:],
                            apack[ch][:, ob * 128:(ob + 1) * 128],
                            wt[:, s - s0],
                            start=(s == s0), stop=(s == s1),
                        )
                x2c = []
                for ob in range(4):
                    yt = x2p.tile([128, FW], BF16, tag=f"x2_{ob}",
                                  name=f"x2_{it}_{dt}_{ob}")
                    if ob < 2:
                        nc.scalar.activation(yt[:], acc[ob][:], PRELU,
                                             bias=0.0, scale=1.0, alpha=0.01)
                    else:
                        nc.vector.tensor_copy(yt[:], acc[ob][:])
                        nc.vector.scalar_tensor_tensor(
                            yt[:], yt[:], 0.01, yt[:],
                            mybir.AluOpType.mult, mybir.AluOpType.max)
                    x2c.append(yt)
                psc = psg.tile([128, FW], F32, tag="g", name=f"psc{it}_{dt}")
                for ob in range(4):
                    nc.tensor.matmul(psc[:], w1t[ob][:], x2c[ob][:],
                                     start=(ob == 0), stop=(ob == 3))
                nc.scalar.activation(
                    pad[:, 1 + dt * DD:1 + (dt + 1) * DD,
                        1 + it * DI:1 + (it + 1) * DI],
                    psc[:].rearrange("p (d t) -> p d t", d=DD),
                    PRELU, bias=b1t, scale=1.0, alpha=0.01)
                # exact zero-padding: the per-core invalid halo column
                # (t = -1 for th0, t = 256 for th1) is scaled by a 0/1 mask
                if it == 0:
                    nc.vector.tensor_tensor(
                        pad[:, 1 + dt * DD:1 + (dt + 1) * DD, 1],
                        pad[:, 1 + dt * DD:1 + (dt + 1) * DD, 1],
                        mlr_t[:, 0:1].to_broadcast([128, DD]),
                        mybir.AluOpType.mult)
                if it == NIT - 1:
                    nc.vector.tensor_tensor(
                        pad[:, 1 + dt * DD:1 + (dt + 1) * DD, TW],
                        pad[:, 1 + dt * DD:1 + (dt + 1) * DD, TW],
                        mlr_t[:, 1:2].to_broadcast([128, DD]),
                        mybir.AluOpType.mult)

            slot_id = {}
            k = 0
            for dt in range(NDT):
                for it in range(NIT):
                    gi, s0, s1 = tiles[(it, dt)]
                    for s in range(s0, s1 + 1):
                        slot_id[(it, dt, s)] = k
                        k += 1

            def stage_d(dc):
                d0 = dc * DCH
                nd = min(DCH, D - d0)
                fw = nd * TW
                psd = psg.tile([128, DCH * TW], F32, tag="d", name=f"psd{dc}")
                for j in range(9):
                    dy, dx = j // 3, j % 3
                    nc.tensor.matmul(
                        psd[:, 0:fw],
                        w2t[j][:],
                        pad[:, d0 + dy:d0 + dy + nd, dx:dx + TW],
                        start=(j == 0), stop=(j == 8),
                    )
                x4c = x4p.tile([128, DCH * TW], BF16, tag=f"x4_{dc}",
                               name=f"x4_{dc}")
                nc.scalar.activation(x4c[:, 0:fw], psd[:, 0:fw], PRELU,
                                     bias=b2t[:], scale=1.0, alpha=0.01)
                x4cs[dc] = x4c

            def stage_e(dc):
                d0 = dc * DCH
                fw = min(DCH, D - d0) * TW
                pse = psg.tile([1, DCH * TW], F32, tag="g", name=f"pse{dc}")
                nc.tensor.matmul(pse[:, 0:fw], w3t[:], x4cs[dc][:, 0:fw],
                                 start=True, stop=True)
                nc.scalar.activation(
                    out_sb[:, d0 * TW:d0 * TW + fw], pse[:, 0:fw], SIG,
                    bias=b3t[:], scale=1.0)

            # d-block 0 tiles, then conv chunks 0..5 interleaved with
            # d-block 1 tiles, then the rest of the conv.
            if emit_rest:
                for it in range(NIT):
                    stage_bc(it, 0)
                for it in range(NIT):
                    stage_bc(it, 1)
                    if trunc != 'bc' and it >= 4 and it % 2 == 0:
                        stage_d(it // 2 - 2)   # dc 0..2 while dt1 runs
                if trunc != 'bc':
                    for dc in range(3, NDCH):
                        stage_d(dc)
                        stage_e(dc - 3)
                    for dc in range(NDCH - 3, NDCH):
                        stage_e(dc)
                else:
                    nc.vector.memset(out_sb[:], 0.0)
                nc.scalar.dma_start(out_d[:], out_sb[:])
    _legalize_waits(nc)
    return nc


_CACHE = {}


def _prep(smp_weight):
    key = hash(smp_weight.tobytes())
    if key in _CACHE:
        return _CACHE[key]
    smp = np.asarray(smp_weight, dtype=np.float32).reshape(T, N, D, T)
    geom = _geometry(smp != 0)
    nc = _build_program(geom)
    core = [_build_core_data(smp, geom, th) for th in range(2)]
    _CACHE[key] = (nc, core)
    return _CACHE[key]


def kernel(feature, smp_weight, w0, b0, w1, b1, w2, b2, w3, b3,
           _trace=False):
    feature = np.asarray(feature, dtype=np.float32)
    nc, core = _prep(np.asarray(smp_weight, dtype=np.float32))

    # w0: [128(c%), N, 2(c-chunk), DIM0]
    w0p = np.ascontiguousarray(
        np.asarray(w0, dtype=np.float32).transpose(2, 1, 0).reshape(
            N, 2, 128, DIM0).transpose(2, 0, 1, 3)).astype(BF)
    w1p = np.asarray(w1, dtype=np.float32).T.reshape(4, 128, DIM1)
    w2p = np.asarray(w2, dtype=np.float32).transpose(2, 3, 1, 0).reshape(
        9, DIM1, DIM1)
    w3p = np.asarray(w3, dtype=np.float32).T                # (128, 1)
    wsm = np.zeros((128, 14 * DIM1 + 1), dtype=np.float32)
    for c in range(4):
        wsm[:, c * DIM1:(c + 1) * DIM1] = w1p[c]
    for j in range(9):
        wsm[:, (4 + j) * DIM1:(5 + j) * DIM1] = w2p[j]
    wsm[:, 13 * DIM1:13 * DIM1 + 1] = w3p
    wsm = wsm.astype(BF)
    b0r = np.asarray(b0, dtype=np.float32).reshape(1, DIM0).astype(BF)
    b123c = []
    for th in range(2):
        a = np.zeros((128, 5), dtype=np.float32)
        a[:, 0] = np.asarray(b1, dtype=np.float32).ravel()
        a[:, 1] = np.asarray(b2, dtype=np.float32).ravel()
        a[0, 2] = float(np.asarray(b3, dtype=np.float32).ravel()[0])
        a[:, 3] = 0.0 if th == 0 else 1.0   # left halo (t=-1 / t=127)
        a[:, 4] = 1.0 if th == 0 else 0.0   # right halo (t=129 / t=256)
        b123c.append(a)

    # feat' per b: [128, 2(c-chunk), 384] covering absolute tau in [-64, 320)
    feats = []
    for b in range(B):
        f = np.zeros((C_IN, AW), dtype=np.float32)
        f[:, -TAU0:-TAU0 + T] = feature[b]
        feats.append(np.ascontiguousarray(
            f.reshape(2, 128, AW).transpose(1, 0, 2)).astype(BF))

    in_maps = []
    for cid in range(8):
        b, th = cid // 2, cid % 2
        idx, wp, cm, _ = core[th]
        in_maps.append({
            "feat": feats[b], "w0t": w0p, "wp": wp, "gidx": idx,
            "wsm": wsm, "b0r": b0r, "b123": b123c[th],
        })
    res = run_bass_kernel_spmd(nc, in_maps, core_ids=list(range(8)),
                               trace=_trace)
    out = np.empty((B, D, T), dtype=np.float32)
    for cid in range(8):
        b, th = cid // 2, cid % 2
        full = res.results[cid]["out"].reshape(D, TW)
        out[b, :, th * 128:(th + 1) * 128] = full[:, 1:TW - 1]
    if _trace:
        return out, res
    return out


# revision 50
# speedup vs baseline: 1.0851x; 1.0075x over previous
"""Trainium2 Bass kernel for nn_BoundaryModule_38422777430159.

Reference computation (B=4, C=256, T=256, N=10, D=40, DIM0=512, DIM1=128):
  x1 = sample(feature)            # (B,C,N,D,T) via (T, N*D*T) smp matmul
  x2 = leaky(einsum('bcndt,ocn->bodt', x1, w0) + b0)
  x3 = leaky(w1 @ x2 + b1)        # 1x1 conv
  x4 = leaky(conv3x3(x3, w2) + b2)
  out = sigmoid(w3 @ x4 + b3)     # (B, D, T)

Device strategy (8 cores SPMD; core = (b = i//2, t-half th = i%2), 1-col halo):

The smp matrix is linear interpolation: each output column (n,d,t) touches
<=2 adjacent tau rows of G_n = w0_n.T @ feature.  The dense contraction
over (n,tau)=2560 rows is therefore ~99% zeros.  This kernel packs, for
each (13 t x 20 d) output tile, exactly the tau-bands it needs into a few
dense 128-row chunks:

  A:    A[n,win] = feat_win.T @ w0_n  (bf16, 3 tau-windows of 128/n)
        -> DMA to a DRAM scratch (3840 rows of [512], +1 row = b0)
  pack: per chunk, one gpsimd indirect-DMA gather (idx per partition,
        host-computed from the smp nonzero pattern); row 0 = b0 row so the
        block0 bias rides the matmul
  B:    per tile: ~2.5 chunks x 4 o-blocks of [128,260] bf16 matmuls
        against host-packed W' slices; Act-engine Prelu -> x2 (bf16)
  C:    x3 = w1 @ x2 (+ b1 x colmask via a rank-1 matmul so halo columns
        stay exactly 0), Prelu written strided into the conv pad buffer
  D/E:  3x3 conv (bf16) + sigmoid(w3 @ x4 + b3) as before.

All matmuls bf16 (1 cyc/row at any free size); PSUM accumulates fp32.
"""
import os
import sys

for _p in ("/opt/trn_rl_repo", "/root/.axon_site/_ro/trn_rl_repo"):
    if os.path.isdir(_p) and _p not in sys.path:
        sys.path.append(_p)

import numpy as np
import ml_dtypes

import concourse.bass as bass
import concourse.tile as tile
from concourse import mybir
from concourse.bass_utils import run_bass_kernel_spmd

T = 256
N = 10
D = 40
B = 4
C_IN = 256
DIM0 = 512
DIM1 = 128

TW = 130            # t-window incl 1-col halo each side
DI, DD = 13, 20     # stage-B tile: 13 t-cols x 20 d-rows
FW = DI * DD        # 260 matmul columns per tile
NIT, NDT = TW // DI, D // DD   # 10 x 2 tiles
KG = 3              # i-tiles grouped per chunk-set
WIN = 3             # tau windows per n in stage A
TAU0 = -64          # window 0 starts at absolute tau = -64
AW = WIN * 128      # 384 feat' columns
NAROW = N * AW      # 3840 real A rows; row 3840 = b0
CAP = 127           # chunk rows 1..127 carry data; row 0 = b0
DCH = 3             # conv d-rows per psum group
NDCH = (D + DCH - 1) // DCH  # 14

F32 = mybir.dt.float32
BF16 = mybir.dt.bfloat16
I32 = mybir.dt.int32
BF = ml_dtypes.bfloat16


def _legalize_waits(nc, limit=1):
    """This walrus build allows a single embedded sync wait per real
    instruction; move the excess onto standalone NoOp wait-carriers."""
    moved = 0
    for f in nc.m.functions:
        for bb in f.blocks:
            il = bb.instructions
            out = []
            changed = False
            for inst in il:
                si = inst.sync_info
                ty = type(inst).__name__
                if (si and si.on_wait and len(si.on_wait) > limit
                        and ty not in ("InstEventSemaphore", "InstNoOp")):
                    keep = si.on_wait[-limit:]
                    for w in si.on_wait[:-limit]:
                        out.append(mybir.InstNoOp(
                            name=f"waitnop-{nc.next_id()}",
                            sync_info=mybir.SyncInfo(on_wait=[w], on_update=[]),
                            bass_nofuse=True,
                            engine=inst.engine,
                        ))
                        moved += 1
                    inst.sync_info = mybir.SyncInfo(
                        on_wait=keep, on_update=si.on_update)
                    changed = True
                out.append(inst)
            if changed:
                bb.instructions = out
    return moved


# ---------------------------------------------------------------------------
# host-side geometry: which (n, tau) rows each stage-B tile needs, grouped
# into shared chunk-sets; identical program structure for every core.
# ---------------------------------------------------------------------------

def _tile_cols(it, th):
    """absolute i for the 13 columns of i-tile `it` on half `th` (may be
    outside [0, T): those columns are pad)."""
    return [it * DI + il - 1 + 128 * th for il in range(DI)]


def _geometry(nzmask):
    """nzmask: [T(tau), N, D, T(i)] bool of the smp matrix.
    Returns the program structure + per-th packing:
      groups: list of dicts with tiles, nch, and per-th
              (chunk_rows[th][c] = list of (n, tau) or None) ...
      tiles:  dict (it,dt) -> (group_idx, s0, s1, b0_slot_local)
    """
    # per (th, it): valid i list and column index
    tile_rows = {}
    for th in range(2):
        for it in range(NIT):
            cols = _tile_cols(it, th)
            valid = [i for i in cols if 0 <= i < T]
            for dt in range(NDT):
                if valid:
                    sub = nzmask[:, :, dt * DD:(dt + 1) * DD, :][:, :, :, valid]
                    tn = np.argwhere(sub.any(axis=(2, 3)))  # (tau, n)
                    rows = set((int(n_), int(t_)) for t_, n_ in tn)
                else:
                    rows = set()
                tile_rows[(th, it, dt)] = rows

    groups = []
    tiles = {}
    chunk_base = 0
    for dt in range(NDT):
        for g0 in range(0, NIT, KG):
            its = list(range(g0, min(g0 + KG, NIT)))
            per_th = []
            spans = [{}, {}]
            for th in range(2):
                sets = [tile_rows[(th, it, dt)] for it in its]
                allrows = sorted(set().union(*sets))
                if allrows:
                    use = {r: [k for k, s in enumerate(sets) if r in s]
                           for r in allrows}
                    allrows.sort(key=lambda r: (float(np.mean(use[r])), r))
                pos = {r: j for j, r in enumerate(allrows)}
                nch = max(1, -(-len(allrows) // CAP))
                chunk_rows = []
                for c in range(nch):
                    chunk_rows.append(allrows[c * CAP:(c + 1) * CAP])
                per_th.append(chunk_rows)
                for it, s in zip(its, sets):
                    if s:
                        ps = [pos[r] for r in s]
                        spans[th][it] = (min(ps) // CAP, max(ps) // CAP)
            nch = max(len(per_th[0]), len(per_th[1]))
            for it in its:
                sp = [spans[th][it] for th in range(2) if it in spans[th]]
                if sp:
                    s0 = min(a for a, b in sp)
                    s1 = max(b for a, b in sp)
                else:
                    s0 = s1 = 0
                tiles[(it, dt)] = (len(groups), s0, s1)
            groups.append(dict(dt=dt, its=its, nch=nch, per_th=per_th,
                               chunk_base=chunk_base))
            chunk_base += nch
    return groups, tiles, chunk_base


def _build_core_data(smp, geom, th):
    """Per-t-half gather indices and packed W' slices (shared across b)."""
    groups, tiles, nch_total = geom
    # smp padded in i: index ip = i+1 in [0, 258)
    smp_pad = np.zeros((T, N, D, T + 2), dtype=np.float32)
    smp_pad[:, :, :, 1:T + 1] = smp

    idx = np.full((128, nch_total), NAROW, dtype=np.int32)  # default: b0 row
    for g in groups:
        rows_c = g["per_th"][th] if th < len(g["per_th"]) else []
        for c in range(g["nch"]):
            rows = rows_c[c] if c < len(rows_c) else []
            for j, (n_, tau) in enumerate(rows):
                idx[1 + j, g["chunk_base"] + c] = n_ * AW + (tau - TAU0)

    # slot list in program emission order: for each tile (dt-major), its
    # slots s0..s1
    slots = []
    for dt in range(NDT):
        for it in range(NIT):
            gi, s0, s1 = tiles[(it, dt)]
            for s in range(s0, s1 + 1):
                slots.append((it, dt, gi, s, s == s0))
    nslot = len(slots)

    wp = np.zeros((nslot, 128, FW), dtype=BF)
    ip_cols = {}
    for it in range(NIT):
        ip_cols[it] = np.array([min(max(i + 1, 0), T + 1)
                                for i in _tile_cols(it, th)])
    for si, (it, dt, gi, s, isfirst) in enumerate(slots):
        g = groups[gi]
        rows_c = g["per_th"][th]
        rows = rows_c[s] if s < len(rows_c) else []
        dsl = slice(dt * DD, (dt + 1) * DD)
        if rows:
            ns = np.array([r[0] for r in rows])
            taus = np.array([r[1] for r in rows])
            vals = smp_pad[taus, ns][:, dsl, :][:, :, ip_cols[it]]
            wp[si, 1:1 + len(rows)] = vals.reshape(len(rows), FW).astype(BF)
        if isfirst:
            mask = np.array([1.0 if 0 <= i < T else 0.0
                             for i in _tile_cols(it, th)], dtype=np.float32)
            wp[si, 0] = np.tile(mask, DD).astype(BF)

    # colmask for the b1 rank-1 matmul, tiled per i-tile: [1, NIT*FW]
    cm = np.zeros((1, NIT * FW), dtype=BF)
    for it in range(NIT):
        mask = np.array([1.0 if 0 <= i < T else 0.0
                         for i in _tile_cols(it, th)], dtype=np.float32)
        cm[0, it * FW:(it + 1) * FW] = np.tile(mask, DD).astype(BF)
    return idx, wp, cm, slots


# ---------------------------------------------------------------------------
# program
# ---------------------------------------------------------------------------

def _build_program(geom, trunc=None):
    # trunc: debug levels 'a' (stage A only), 'pack', 'bc' (no conv)
    groups, tiles, nch_total = geom
    slots = []
    for dt in range(NDT):
        for it in range(NIT):
            gi, s0, s1 = tiles[(it, dt)]
            for s in range(s0, s1 + 1):
                slots.append((it, dt, gi, s))
    nslot = len(slots)
    WMAX = max(s1 - s0 + 1 for _, s0, s1 in tiles.values())

    nc = bass.Bass(trn_type="TRN2", num_swdge_queues=4)
    PRELU = mybir.ActivationFunctionType.Prelu
    SIG = mybir.ActivationFunctionType.Sigmoid

    feat_d = nc.dram_tensor("feat", [128, 2, AW], BF16, kind="ExternalInput")
    w0_d = nc.dram_tensor("w0t", [128, N, 2, DIM0], BF16, kind="ExternalInput")
    wp_d = nc.dram_tensor("wp", [nslot, 128, FW], BF16, kind="ExternalInput")
    idx_d = nc.dram_tensor("gidx", [128, nch_total], I32, kind="ExternalInput")
    # w1 (4x128) | w2 (9x128) | w3 (1 col) along the free dim
    wsm_d = nc.dram_tensor("wsm", [128, 14 * DIM1 + 1], BF16,
                           kind="ExternalInput")
    b0r_d = nc.dram_tensor("b0r", [1, DIM0], BF16, kind="ExternalInput")
    # per-partition columns: b1 | b2 | b3(row0) | halo maskL | maskR
    b123_d = nc.dram_tensor("b123", [128, 5], F32, kind="ExternalInput")
    a_d = nc.dram_tensor("adram", [NAROW + 1, DIM0], BF16, kind="Internal")
    out_d = nc.dram_tensor("out", [1, D * TW], F32, kind="ExternalOutput")

    with tile.TileContext(nc) as tc:
        with (
            tc.tile_pool(name="inp", bufs=1) as inp,
            tc.tile_pool(name="asb", bufs=4) as asb,
            tc.tile_pool(name="apk", bufs=1) as apk,
            tc.tile_pool(name="wst", bufs=4) as wst,
            tc.tile_pool(name="x2p", bufs=2) as x2p,
            tc.tile_pool(name="x3p", bufs=1) as x3p,
            tc.tile_pool(name="x4p", bufs=2) as x4p,
            tc.tile_pool(name="outp", bufs=1) as outp,
            tc.tile_pool(name="psb", bufs=1, space="PSUM") as psb,
            tc.tile_pool(name="psg", bufs=2, space="PSUM") as psg,
        ):
            # ---- input DMAs (few, large; feat + first w0 group first) ----
            ft = inp.tile([128, 2, AW], BF16, tag="f", name="feat_sb")
            nc.sync.dma_start(ft[:], feat_d[:])
            feat = [ft[:, c] for c in range(2)]
            w0sb = inp.tile([128, N, 2, DIM0], BF16, tag="w0", name="w0_sb")
            NG = [(0, 3), (3, 6), (6, 9), (9, 10)]
            for gi_, (n0, n1) in enumerate(NG):
                eng = (nc.sync, nc.scalar)[gi_ % 2]
                eng.dma_start(w0sb[:, n0:n1], w0_d[:, n0:n1])
            w0t = [[w0sb[:, n, c] for c in range(2)] for n in range(N)]
            b0r_t = inp.tile([1, DIM0], BF16, tag="b0r", name="b0r_sb")
            nc.scalar.dma_start(b0r_t[:], b0r_d[:])
            # b0 row of the A scratch
            nc.scalar.dma_start(a_d[NAROW:NAROW + 1, :], b0r_t[:])
            idx_t = inp.tile([128, nch_total], I32, tag="idx", name="gidx_sb")
            nc.sync.dma_start(idx_t[:], idx_d[:])
            wsm = inp.tile([128, 14 * DIM1 + 1], BF16, tag="wsm", name="wsm_sb")
            nc.scalar.dma_start(wsm[:], wsm_d[:])
            w1t = [wsm[:, c * DIM1:(c + 1) * DIM1] for c in range(4)]
            w2t = [wsm[:, (4 + j) * DIM1:(5 + j) * DIM1] for j in range(9)]
            w3t = wsm[:, 13 * DIM1:13 * DIM1 + 1]
            b123 = inp.tile([128, 5], F32, tag="b123", name="b123_sb")
            nc.sync.dma_start(b123[:], b123_d[:])
            b1t = b123[:, 0:1]
            b2t = b123[:, 1:2]
            b3t = b123[0:1, 2:3]
            mlr_t = inp.tile([128, 2], BF16, tag="mlr", name="mlr_sb")
            nc.vector.tensor_copy(mlr_t[:], b123[:, 3:5])

            # ---- PE warm-up: keep the PE busy (and its p-state ramped)
            # while the feat/w0 DMAs land and later while the gathers run.
            warm = inp.tile([128, DIM0], BF16, tag="wm", name="warm_sb")
            nc.vector.memset(warm[:].bitcast(mybir.dt.uint16), 0)
            wps = psb.tile([1, DIM0], F32, tag="b3", name="warm_ps")

            def warm_mm(k, free=64):
                nc.tensor.matmul(wps[:, 0:free], warm[:, 0:1], warm[:, 0:free],
                                 start=True, stop=True)

            for k in range(10):
                warm_mm(k)

            # ---- stage A: A[n,win] = feat_win.T @ w0_n -> adram ----
            for n in range(N):
                a_n = asb.tile([128, WIN, DIM0], BF16, tag=f"a{n % 4}",
                               name=f"a{n}")
                for win in range(WIN):
                    k = n * WIN + win
                    ps = psb.tile([128, DIM0], F32, tag=f"b{k % 4}",
                                  name=f"psa{n}_{win}")
                    for c in range(2):
                        nc.tensor.matmul(
                            ps[:],
                            feat[c][:, win * 128:(win + 1) * 128],
                            w0t[n][c][:],
                            start=(c == 0), stop=(c == 1),
                        )
                    if (n * WIN + win) % 2:
                        nc.vector.tensor_copy(a_n[:, win], ps[:])
                    else:
                        nc.scalar.activation(
                            a_n[:, win], ps[:],
                            mybir.ActivationFunctionType.Copy,
                            bias=0.0, scale=1.0)
                nc.sync.dma_start(
                    a_d[n * AW:(n + 1) * AW, :].rearrange(
                        "(w p) e -> p w e", p=128),
                    a_n[:])

            # ---- pack: one indirect gather per chunk ----
            apack = []
            if trunc != 'a':
                for ch in range(nch_total):
                    g = apk.tile([128, DIM0], BF16, tag=f"ap{ch}", name=f"ap{ch}")
                    h = nc.gpsimd.indirect_dma_start(
                        out=g[:], out_offset=None, in_=a_d[:],
                        in_offset=bass.IndirectOffsetOnAxis(
                            ap=idx_t[:, ch:ch + 1], axis=0))
                    q = ch % 4
                    if q:
                        h.ins.queue = f"qPoolDynamic{q}"
                    apack.append(g)

            if trunc in ('a', 'pack'):
                out_sb = outp.tile([1, D * TW], F32, tag="os", name="out_sb")
                nc.vector.memset(out_sb[:], 0.0)
                for ch in range(len(apack)):
                    nc.vector.tensor_copy(out_sb[:, ch:ch + 1],
                                          apack[ch][0:1, 0:1].bitcast(BF16)[:, 0:1] if False
                                          else apack[ch][0:1, 0:1])
                nc.scalar.dma_start(out_d[:], out_sb[:])

            # ---- stages B+C per tile; conv chunks interleaved per d-block --
            emit_rest = trunc not in ('a', 'pack')
            pad = x3p.tile([128, D + 2, TW + 2], BF16, tag="pad", name="padbuf")
            if emit_rest:
                nc.vector.memset(pad[:].bitcast(mybir.dt.uint16), 0)
                out_sb = outp.tile([1, D * TW], F32, tag="os", name="out_sb")
            x4cs = [None] * NDCH

            def stage_bc(it, dt):
                gi, s0, s1 = tiles[(it, dt)]
                g = groups[gi]
                ns = s1 - s0 + 1
                si0 = slot_id[(it, dt, s0)]
                wt = wst.tile([128, WMAX, FW], BF16, tag="w",
                              name=f"wp{it}_{dt}")
                eng = (nc.sync, nc.scalar)[(it + dt) % 2]
                eng.dma_start(wt[:, 0:ns],
                              wp_d[si0:si0 + ns].transpose((1, 0, 2)))
                acc = [psb.tile([128, FW], F32, tag=f"b{ob}",
                                name=f"psb{it}_{dt}_{ob}") for ob in range(4)]
                for s in range(s0, s1 + 1):
                    ch = g["chunk_base"] + s
                    for ob in range(4):
                        nc.tensor.matmul(
                            acc[ob][:],
                            apack[ch][:, ob * 128:(ob + 1) * 128],
                            wt[:, s - s0],
                            start=(s == s0), stop=(s == s1),
                        )
                x2c = []
                for ob in range(4):
                    yt = x2p.tile([128, FW], BF16, tag=f"x2_{ob}",
                                  name=f"x2_{it}_{dt}_{ob}")
                    if ob < 2:
                        nc.scalar.activation(yt[:], acc[ob][:], PRELU,
                                             bias=0.0, scale=1.0, alpha=0.01)
                    else:
                        nc.vector.tensor_copy(yt[:], acc[ob][:])
                        nc.vector.scalar_tensor_tensor(
                            yt[:], yt[:], 0.01, yt[:],
                            mybir.AluOpType.mult, mybir.AluOpType.max)
                    x2c.append(yt)
                psc = psg.tile([128, FW], F32, tag="g", name=f"psc{it}_{dt}")
                for ob in range(4):
                    nc.tensor.matmul(psc[:], w1t[ob][:], x2c[ob][:],
                                     start=(ob == 0), stop=(ob == 3))
                nc.scalar.activation(
                    pad[:, 1 + dt * DD:1 + (dt + 1) * DD,
                        1 + it * DI:1 + (it + 1) * DI],
                    psc[:].rearrange("p (d t) -> p d t", d=DD),
                    PRELU, bias=b1t, scale=1.0, alpha=0.01)
                # exact zero-padding: the per-core invalid halo column
                # (t = -1 for th0, t = 256 for th1) is scaled by a 0/1 mask
                if it == 0:
                    nc.vector.tensor_tensor(
                        pad[:, 1 + dt * DD:1 + (dt + 1) * DD, 1],
                        pad[:, 1 + dt * DD:1 + (dt + 1) * DD, 1],
                        mlr_t[:, 0:1].to_broadcast([128, DD]),
                        mybir.AluOpType.mult)
                if it == NIT - 1:
                    nc.vector.tensor_tensor(
                        pad[:, 1 + dt * DD:1 + (dt + 1) * DD, TW],
                        pad[:, 1 + dt * DD:1 + (dt + 1) * DD, TW],
                        mlr_t[:, 1:2].to_broadcast([128, DD]),
                        mybir.AluOpType.mult)

            slot_id = {}
            k = 0
            for dt in range(NDT):
                for it in range(NIT):
                    gi, s0, s1 = tiles[(it, dt)]
                    for s in range(s0, s1 + 1):
                        slot_id[(it, dt, s)] = k
                        k += 1

            def stage_d(dc):
                d0 = dc * DCH
                nd = min(DCH, D - d0)
                fw = nd * TW
                psd = psg.tile([128, DCH * TW], F32, tag="d", name=f"psd{dc}")
                for j in range(9):
                    dy, dx = j // 3, j % 3
                    nc.tensor.matmul(
                        psd[:, 0:fw],
                        w2t[j][:],
                        pad[:, d0 + dy:d0 + dy + nd, dx:dx + TW],
                        start=(j == 0), stop=(j == 8),
                    )
                x4c = x4p.tile([128, DCH * TW], BF16, tag=f"x4_{dc}",
                               name=f"x4_{dc}")
                nc.scalar.activation(x4c[:, 0:fw], psd[:, 0:fw], PRELU,
                                     bias=b2t[:], scale=1.0, alpha=0.01)
                x4cs[dc] = x4c

            def stage_e(dc):
                d0 = dc * DCH
                fw = min(DCH, D - d0) * TW
                pse = psg.tile([1, DCH * TW], F32, tag="g", name=f"pse{dc}")
                nc.tensor.matmul(pse[:, 0:fw], w3t[:], x4cs[dc][:, 0:fw],
                                 start=True, stop=True)
                nc.scalar.activation(
                    out_sb[:, d0 * TW:d0 * TW + fw], pse[:, 0:fw], SIG,
                    bias=b3t[:], scale=1.0)

            # d-block 0 tiles, then conv chunks 0..5 interleaved with
            # d-block 1 tiles, then the rest of the conv.
            if emit_rest:
                for it in range(NIT):
                    stage_bc(it, 0)
                for it in range(NIT):
                    stage_bc(it, 1)
                    if trunc != 'bc':
                        # conv chunks 0..5 touch only d-block-0 pad rows, so
                        # they can fill PE gather-wait gaps during dt1
                        if it % 2 == 0:
                            stage_d(it // 2)   # dc 0..4 while dt1 runs
                        elif it == NIT - 1:
                            stage_d(5)
                if trunc != 'bc':
                    for dc in range(6, NDCH):
                        stage_d(dc)
                        stage_e(dc - 6)
                    for dc in range(NDCH - 6, NDCH):
                        stage_e(dc)
                else:
                    nc.vector.memset(out_sb[:], 0.0)
                nc.scalar.dma_start(out_d[:], out_sb[:])
    _legalize_waits(nc)
    return nc


_CACHE = {}


def _prep(smp_weight):
    key = hash(smp_weight.tobytes())
    if key in _CACHE:
        return _CACHE[key]
    smp = np.asarray(smp_weight, dtype=np.float32).reshape(T, N, D, T)
    geom = _geometry(smp != 0)
    nc = _build_program(geom)
    core = [_build_core_data(smp, geom, th) for th in range(2)]
    _CACHE[key] = (nc, core)
    return _CACHE[key]


def kernel(feature, smp_weight, w0, b0, w1, b1, w2, b2, w3, b3,
           _trace=False):
    feature = np.asarray(feature, dtype=np.float32)
    nc, core = _prep(np.asarray(smp_weight, dtype=np.float32))

    # w0: [128(c%), N, 2(c-chunk), DIM0]
    w0p = np.ascontiguousarray(
        np.asarray(w0, dtype=np.float32).transpose(2, 1, 0).reshape(
            N, 2, 128, DIM0).transpose(2, 0, 1, 3)).astype(BF)
    w1p = np.asarray(w1, dtype=np.float32).T.reshape(4, 128, DIM1)
    w2p = np.asarray(w2, dtype=np.float32).transpose(2, 3, 1, 0).reshape(
        9, DIM1, DIM1)
    w3p = np.asarray(w3, dtype=np.float32).T                # (128, 1)
    wsm = np.zeros((128, 14 * DIM1 + 1), dtype=np.float32)
    for c in range(4):
        wsm[:, c * DIM1:(c + 1) * DIM1] = w1p[c]
    for j in range(9):
        wsm[:, (4 + j) * DIM1:(5 + j) * DIM1] = w2p[j]
    wsm[:, 13 * DIM1:13 * DIM1 + 1] = w3p
    wsm = wsm.astype(BF)
    b0r = np.asarray(b0, dtype=np.float32).reshape(1, DIM0).astype(BF)
    b123c = []
    for th in range(2):
        a = np.zeros((128, 5), dtype=np.float32)
        a[:, 0] = np.asarray(b1, dtype=np.float32).ravel()
        a[:, 1] = np.asarray(b2, dtype=np.float32).ravel()
        a[0, 2] = float(np.asarray(b3, dtype=np.float32).ravel()[0])
        a[:, 3] = 0.0 if th == 0 else 1.0   # left halo (t=-1 / t=127)
        a[:, 4] = 1.0 if th == 0 else 0.0   # right halo (t=129 / t=256)
        b123c.append(a)

    # feat' per b: [128, 2(c-chunk), 384] covering absolute tau in [-64, 320)
    feats = []
    for b in range(B):
        f = np.zeros((C_IN, AW), dtype=np.float32)
        f[:, -TAU0:-TAU0 + T] = feature[b]
        feats.append(np.ascontiguousarray(
            f.reshape(2, 128, AW).transpose(1, 0, 2)).astype(BF))

    in_maps = []
    for cid in range(8):
        b, th = cid // 2, cid % 2
        idx, wp, cm, _ = core[th]
        in_maps.append({
            "feat": feats[b], "w0t": w0p, "wp": wp, "gidx": idx,
            "wsm": wsm, "b0r": b0r, "b123": b123c[th],
        })
    res = run_bass_kernel_spmd(nc, in_maps, core_ids=list(range(8)),
                               trace=_trace)
    out = np.empty((B, D, T), dtype=np.float32)
    for cid in range(8):
        b, th = cid // 2, cid % 2
        full = res.results[cid]["out"].reshape(D, TW)
        out[b, :, th * 128:(th + 1) * 128] = full[:, 1:TW - 1]
    if _trace:
        return out, res
    return out
